# revision 10
# baseline (speedup 1.0000x reference)
"""Bahdanau-attention scores kernel for Trainium2 (8 NeuronCores, SPMD).

Computation (per batch row b):
    pre[s, k] = hidden[b] @ Wh + enc[b, s] @ We + b_attn       (S=1024, E=K=1024)
    scores[s] = tanh(pre[s, :]) @ v
    out[b]    = softmax(where(mask[b]==0, -1e10, scores))      over s

Sharding: data-parallel over batch B=64 -> 8 batches per core; weights
replicated. No collectives.

Per-core structure (fp8 DoubleRow main matmul, bf16 elsewhere):
  - enc HBM->fp8-pair pipeline runs entirely on HWDGE rings + DVE, with NO
    SWDGE DMAs in flight: Tile fences xbar transposes against every
    in-flight SWDGE DMA (both directions, all 8 DMASW queues), which in the
    previous DRAM->DRAM SWDGE-cast design serialized cast(13.4us) ->
    tp-issue(4.2) -> tp(5.2) into the 23.9us/batch critical path.  Now:
      scalar ring: enc f32 DRAM->SBUF loads (natural [p=s%128, st, e])
      DVE:         tensor_copy f32 -> fp8e4 (adjacent-e pairs land in
                   adjacent bytes = the DoubleRow rhs pairing)
      sync ring:   fp8 SBUF->DRAM bounce store, then DRAM->SBUF xbar
                   transpose of the u16 pairs: encT8h[p, et, s]
    HWDGE traffic/batch = 4MB(load)+1MB(store)+1MB(tp) = 6MB at the
    ~358 GB/s per-core cap = 16.8us ~= the PE's ~17us/batch: true ridge.
  - one DRAM bounce tensor per (batch, half) so no coarse DRAM-range
    tracking can serialize different batches' stores/transposes.
  - main MM: pre[k, s] = sum_et lhsT(w8) @ rhs(encT8h), DoubleRow, one
    LDWEIGHTS per (et, kt) serving both s-halves.
  - ScalarE: tanh(psum/64 + (hidden@Wh + b_attn)[k]) -> SBUF bf16
  - hidden@Wh (hp) is interleaved per-kt into batch 0's loop (wh loaded as
    per-kt chunks) so it doesn't sit at the PE FIFO head blocking the
    first main MMs behind a 2MB weight load.
  - v-dot: 4 col-tiled PE matmuls (tile_position=(0,32q)), lag THREE
    k-tiles behind the main MMs (ACT falls ~1.5 groups behind the PE by
    batch end; lag 2 stalled the PE 0.5-1.5us per batch).  Last 3 k-tiles
    carried into the next batch, one k-tile per MM group, all 4 quarter
    MMs before the 4 flat4 copies (interleaving MMs and copies created
    false column-range WARs = 3x850ns PE bubbles).
  - softmax in chunks: batches 0-5 during iter 7, 6-7 at the end; gathers
    flat4->scores are HWDGE SBUF->SBUF (no accum; mask offset added on
    DVE), so they don't fence against transposes either.

Sync note: this walrus build encodes at most ONE semaphore wait per
instruction; _split_multi_waits() rewrites Tile's multi-wait instructions
into NoOp(wait) chains on the same engine.
"""

import sys

if "/opt/trn_rl_repo" not in sys.path:
    sys.path.insert(0, "/opt/trn_rl_repo")

from contextlib import ExitStack

import numpy as np

B, S, E, K = 64, 1024, 1024, 1024  # E = 2*ENC_HID, K = DEC_HID
NCORES = 8
BL = B // NCORES  # batches per core
NEG = -1e10
WSCALE = 64.0     # We quantization scale into E4M3 range

ET2 = E // 256  # 4 DoubleRow e-tiles (256-deep contraction each)
KT = K // 128   # 8 k-tiles
ST = S // 128   # 8 s-tiles
NB = 512        # matmul free-dim block (one s-half)
SB = S // NB    # 2 s-halves
VLAG = 3        # v-dot lag in k-tiles behind the main MMs

_CACHE = {}


def _build_bass(strip=True):
    from concourse import bass, mybir, tile

    f32 = mybir.dt.float32
    bf16 = mybir.dt.bfloat16
    f8 = mybir.dt.float8e4
    u16 = mybir.dt.uint16
    i32 = mybir.dt.int32
    Tanh = mybir.ActivationFunctionType.Tanh
    Exp = mybir.ActivationFunctionType.Exp
    Alu = mybir.AluOpType
    Ax = mybir.AxisListType
    DR = mybir.MatmulPerfMode.DoubleRow

    nc = bass.Bass()

    enc_d = nc.declare_dram_parameter("encoder_outputs", [BL, S, E], f32, isOutput=False)
    # one bounce tensor per (batch, half): no shared-tensor dep tracking
    enc8_d = [[nc.dram_tensor(f"enc8_{b}_{h}", [NB, E], f8) for h in range(SB)]
              for b in range(BL)]
    mask_d = nc.declare_dram_parameter("mask", [BL, S], i32, isOutput=False)
    wh_d = nc.declare_dram_parameter("wh_pack", [128, KT, K], bf16, isOutput=False)
    w8_d = nc.declare_dram_parameter("w8_pack", [128, ET2, 2, K], f8, isOutput=False)
    hT_d = nc.declare_dram_parameter("hT_pack", [128, KT * BL], bf16, isOutput=False)
    b_d = nc.declare_dram_parameter("b_pack", [1, K], bf16, isOutput=False)
    v_d = nc.declare_dram_parameter("v_pack", [128, KT], bf16, isOutput=False)
    out_d = nc.declare_dram_parameter("out", [BL, S], f32, isOutput=True)

    with tile.TileContext(nc) as tc, ExitStack() as ctx:
        const = ctx.enter_context(tc.tile_pool(name="const", bufs=1))
        f32_pool = ctx.enter_context(tc.tile_pool(name="encf32", bufs=4))
        f8_pool = ctx.enter_context(tc.tile_pool(name="encf8", bufs=3))
        tp_pool = ctx.enter_context(tc.tile_pool(name="encT", bufs=6))
        tanh_pool = ctx.enter_context(tc.tile_pool(name="tanh", bufs=5))
        pre_ps = ctx.enter_context(tc.tile_pool(name="pre_ps", bufs=3, space="PSUM"))
        sc_ps = ctx.enter_context(tc.tile_pool(name="sc_ps", bufs=1, space="PSUM"))
        fin = ctx.enter_context(tc.tile_pool(name="fin", bufs=1))

        # ---- weights (host-packed) ----
        # w8 gates the first main matmul: first on the sync ring.
        w8 = const.tile([128, ET2, 2, K], f8)
        nc.sync.dma_start(w8[:], w8_d[:])
        hT_bf = const.tile([128, KT * BL], bf16)
        nc.sync.dma_start(hT_bf[:], hT_d[:])
        b_attn_bf = const.tile([1, K], bf16)
        nc.sync.dma_start(b_attn_bf[:], b_d[:])
        v_bf = const.tile([128, KT], bf16)
        nc.sync.dma_start(v_bf[:], v_d[:])
        # wh per-kt chunks on the scalar ring (repacked kt-major on host)
        wh_bf = const.tile([128, KT, KT, 128], bf16)  # [p, kt, dt, c]

        ones_bf = const.tile([1, BL], bf16)
        nc.vector.memset(ones_bf[:], 1.0)

        def stage_load(b, h):
            """enc f32 half-batch -> SBUF [p, st, e], s = st*128 + p."""
            t = f32_pool.tile([128, ST // SB, E], f32, tag="encf32",
                              name=f"encf32_{b}_{h}")
            nc.scalar.dma_start(
                t[:], enc_d[b, h * NB:(h + 1) * NB, :].rearrange(
                    "(st p) e -> p st e", p=128))
            return t

        def stage_cast(t32, b, h):
            t8 = f8_pool.tile([128, ST // SB, E], f8, tag="encf8",
                              name=f"encf8_{b}_{h}")
            nc.vector.tensor_copy(t8[:], t32[:])
            return t8

        def stage_store(t8, b, h):
            nc.sync.dma_start(
                enc8_d[b][h][:].rearrange("(st p) e -> p st e", p=128), t8[:])

        def stage_tp(b, h):
            """DRAM->SBUF u16-pair xbar transpose of the fp8 bounce half:
            encT8h u16[p, et, s] = fp8 pair
            (enc[b, h*512 + s, et*256+2p], enc[b, h*512 + s, et*256+2p+1])."""
            eh = tp_pool.tile([128, ET2, NB], u16, tag="encT",
                              name=f"encT_{b}_{h}")
            nc.sync.dma_start(eh[:], enc8_d[b][h][:].bitcast(u16),
                              transpose=True)
            return eh

        def rhs_view(eh, f8dt):
            return eh[:].bitcast(f8dt).rearrange("p et (s j) -> p et j s", j=2)

        hpb = const.tile([128, KT * BL], f32)  # col = kt*BL + b

        def emit_hp_kt(kt):
            # h_proj[k, b] = sum_d Wh[d, k]*hidden[b, d] + b_attn[k],
            # one k-tile at a time, interleaved into batch 0's MM stream
            hp_ps = pre_ps.tile([128, NB], f32, tag="pre", name=f"hp_ps{kt}",
                                bufs=7)
            for dt in range(KT):
                nc.tensor.matmul(
                    hp_ps[:, :BL],
                    wh_bf[:, kt, dt, :],
                    hT_bf[:, dt * BL:(dt + 1) * BL],
                    start=(dt == 0),
                    stop=False,
                )
            nc.tensor.matmul(
                hp_ps[:, :BL],
                b_attn_bf[:, kt * 128:(kt + 1) * 128],
                ones_bf[:],
                start=False,
                stop=True,
            )
            # ACT, not DVE: the DVE is busy with next-batch casts at b0
            # start and the first tanh would block behind them (gpsimd
            # cannot read PSUM)
            nc.scalar.copy(hpb[:, kt * BL:(kt + 1) * BL], hp_ps[:, :BL])

        # scores accumulate on PSUM rows 32q (col-group q = s-quarter q);
        # staged in flat4 rows 32q on SBUF, gathered to [b, S] chunks
        flat4 = fin.tile([97, BL * 256], f32)

        # softmax state
        mask_i = fin.tile([BL, S], i32)
        maskoff = fin.tile([BL, S], f32)
        scores = fin.tile([BL, S], f32)
        scm = fin.tile([BL, S], f32)
        negmax = fin.tile([BL, 1], f32)
        expv = fin.tile([BL, S], f32)
        rowsum = fin.tile([BL, 1], f32)
        recip = fin.tile([BL, 1], f32)
        outf = fin.tile([BL, S], f32)

        def emit_mask_prep():
            nc.scalar.dma_start(mask_i[:], mask_d[:])
            nc.vector.tensor_copy(maskoff[:], mask_i[:])
            nc.vector.tensor_scalar(
                maskoff[:], maskoff[:], -NEG, NEG, Alu.mult, Alu.add)

        def emit_softmax_chunk(b0, b1):
            # gather flat4 quarter-rows into [b, s] layout (HWDGE SBUF->SBUF)
            for q in range(4):
                nc.scalar.dma_start(
                    scores[b0:b1, q * 256:(q + 1) * 256],
                    flat4[32 * q:32 * q + 1, b0 * 256:b1 * 256])
            # compute ops always span [0, b1): DVE/ACT partition windows
            # must start at 0 (quadrant alignment); recomputing the already
            # finished low batches is idempotent and partition-parallel
            nc.vector.tensor_add(
                scm[:b1, :], scores[:b1, :], maskoff[:b1, :])
            nc.vector.tensor_reduce(
                negmax[:b1], scm[:b1, :], Ax.X, Alu.max, negate=True)
            nc.scalar.activation(
                expv[:b1, :], scm[:b1, :], Exp, bias=negmax[:b1],
                accum_out=rowsum[:b1])
            nc.vector.reciprocal(recip[:b1], rowsum[:b1])
            nc.vector.tensor_scalar_mul(
                outf[:b1, :], expv[:b1, :], recip[:b1])
            nc.scalar.dma_start(out_d[b0:b1, :], outf[b0:b1, :])

        # ---- prologue ----
        t32 = {}
        t32[(0, 0)] = stage_load(0, 0)
        t32[(0, 1)] = stage_load(0, 1)
        # wh chunks follow b0's loads on the scalar ring
        for ck in range(KT):
            # wh_d dims are [p, kt, (dt c)] after the host kt-major repack
            nc.scalar.dma_start(wh_bf[:, ck], wh_d[:, ck])
        t32[(1, 0)] = stage_load(1, 0)
        t32[(1, 1)] = stage_load(1, 1)
        encTs = {}

        def stage_batch(b):
            # casts, then both stores, then both tps: a tp between the
            # stores would head-of-line-block the second store's ring push
            t8s = [stage_cast(t32.pop((b, h)), b, h) for h in range(SB)]
            for h in range(SB):
                stage_store(t8s[h], b, h)
            for h in range(SB):
                encTs[(b, h)] = stage_tp(b, h)

        stage_batch(0)

        # scores PSUM: ONE bank, halves alternated by batch parity
        scband = sc_ps.tile([128, 2, 256], f32, tag="sc", name="scband")

        def scq(b, q):
            return scband[32 * q:32 * q + 1, b % 2, :]

        carry = []  # [(b_prev, kt, th_tile, col_off, qs)] not yet emitted

        def emit_vdots(b, kt, th, col_off, qs, stop):
            for q in qs:
                nc.tensor.matmul(
                    scq(b, q),
                    v_bf[:, kt:kt + 1],
                    th[:, col_off + (q - qs[0]) * 256:
                       col_off + (q - qs[0] + 1) * 256],
                    start=(kt == 0), stop=stop,
                    tile_position=(0, 32 * q))
            if stop:
                # all MMs above before any copy: interleaving creates false
                # column-range WARs that bubble the PE ~850ns per quarter
                for q in qs:
                    nc.vector.tensor_copy(
                        flat4[32 * q:32 * q + 1, b * 256:(b + 1) * 256],
                        scq(b, q))

        def emit_carry(budget):
            while carry and budget > 0:
                b_p, kt, th, col_off, qs = carry.pop(0)
                emit_vdots(b_p, kt, th, col_off, qs, stop=(kt == KT - 1))
                budget -= 1

        # ---- main loop over local batches (software-pipelined) ----
        for b in range(BL):
            # stage next batches: loads(b+2) on scalar; cast+store+tp (b+1)
            # on DVE + sync ring
            if b + 2 < BL:
                t32[(b + 2, 0)] = stage_load(b + 2, 0)
                t32[(b + 2, 1)] = stage_load(b + 2, 1)
            if b + 1 < BL:
                stage_batch(b + 1)
            if b == 1:
                emit_mask_prep()

            if b == 0:
                # sb-major: start on the first transposed s-half immediately;
                # hp(kt) interleaved into the sb0 pass
                ths = {}
                for sb in range(SB):
                    rh = rhs_view(encTs.pop((0, sb)), f8)
                    for kt in range(KT):
                        pre = pre_ps.tile([128, NB], f32, tag="pre",
                                          name="preh", bufs=7)
                        for et in range(ET2):
                            nc.tensor.matmul(
                                pre[:],
                                w8[:, et, :, kt * 128:(kt + 1) * 128],
                                rh[:, et, :, :],
                                start=(et == 0),
                                stop=(et == ET2 - 1),
                                perf_mode=DR,
                            )
                        if sb == 0:
                            emit_hp_kt(kt)
                        th = tanh_pool.tile([128, NB], bf16, tag="thh",
                                            name="thh", bufs=5)
                        nc.scalar.activation(
                            th[:], pre[:], Tanh,
                            bias=hpb[:, kt * BL:kt * BL + 1],
                            scale=1.0 / WSCALE,
                        )
                        ths[(sb, kt)] = th
                        if kt >= 1:
                            emit_vdots(0, kt - 1, ths[(sb, kt - 1)], 0,
                                       (2 * sb, 2 * sb + 1), stop=False)
                    carry.append((0, KT - 1, ths[(sb, KT - 1)], 0,
                                  (2 * sb, 2 * sb + 1)))
            else:
                rhA = rhs_view(encTs.pop((b, 0)), f8)
                rhB = rhs_view(encTs.pop((b, 1)), f8)
                ths = {}
                for kt in range(KT):
                    pres = [pre_ps.tile([128, NB], f32, tag="pre",
                                        name=f"pre{sb}", bufs=7)
                            for sb in range(SB)]
                    for et in range(ET2):  # one LDWEIGHTS serves both sb
                        for sb, rh in ((0, rhA), (1, rhB)):
                            nc.tensor.matmul(
                                pres[sb][:],
                                w8[:, et, :, kt * 128:(kt + 1) * 128],
                                rh[:, et, :, :],
                                start=(et == 0),
                                stop=(et == ET2 - 1),
                                perf_mode=DR,
                            )
                    if kt < VLAG:
                        emit_carry(1)
                    th = tanh_pool.tile([128, SB * NB], bf16, tag="tanh",
                                        bufs=5)
                    for sb in range(SB):
                        nc.scalar.activation(
                            th[:, sb * NB:(sb + 1) * NB], pres[sb][:], Tanh,
                            bias=hpb[:, kt * BL + b:kt * BL + b + 1],
                            scale=1.0 / WSCALE,
                        )
                    ths[kt] = th
                    if kt >= VLAG:
                        emit_vdots(b, kt - VLAG, ths[kt - VLAG], 0,
                                   (0, 1, 2, 3), stop=False)
                for kt in range(KT - VLAG, KT):
                    carry.append((b, kt, ths[kt], 0, (0, 1, 2, 3)))
            if b == 7:
                emit_softmax_chunk(0, 6)

        emit_carry(len(carry))
        emit_softmax_chunk(6, 8)

    if strip:
        _split_multi_waits(nc, mybir)
    return nc


def _split_multi_waits(nc, mybir):
    """Move extra semaphore waits onto standalone NoOps on the same engine.

    This walrus build encodes at most one sync-wait command per instruction,
    but Tile emits instructions with several (cross-engine RAW + WAR + DMA
    queue ordering). A NoOp carrying one wait, placed immediately before the
    instruction in the same engine's stream, is semantically identical: the
    engine's sequencer blocks on the NoOp's wait before dispatching the real
    instruction.
    """
    n = 0
    for fn in nc.m.functions:
        for blk in fn.blocks:
            insts = blk.instructions
            new = []
            changed = False
            for inst in insts:
                si = inst.sync_info
                if si is not None and si.on_wait and len(si.on_wait) > 1:
                    for w in list(si.on_wait)[:-1]:
                        n += 1
                        new.append(mybir.InstNoOp(
                            name=f"{inst.name}-sw{n}",
                            engine=inst.engine,
                            text_hint="split_wait",
                            bass_nofuse=True,
                            sync_info=mybir.SyncInfo(
                                on_wait=[w], on_update=[]),
                        ))
                    inst.sync_info = mybir.SyncInfo(
                        on_wait=[list(si.on_wait)[-1]],
                        on_update=list(si.on_update or []))
                    changed = True
                new.append(inst)
            if changed:
                blk.instructions = new


def get_nc(strip=True):
    key = ("nc", strip)
    if key not in _CACHE:
        _CACHE[key] = _build_bass(strip)
    return _CACHE[key]


def make_in_maps(hidden, encoder_outputs, mask, W_attn, b_attn, v):
    import ml_dtypes

    bf16 = ml_dtypes.bfloat16
    f8 = ml_dtypes.float8_e4m3

    W_attn = np.asarray(W_attn, dtype=np.float32)
    Wh, We = W_attn[:K], W_attn[K:]
    # wh_pack[p, kt, dt, c] = Wh[dt*128 + p, kt*128 + c]  (kt-major chunks)
    wh_pack = np.ascontiguousarray(
        Wh.reshape(KT, 128, KT, 128).transpose(1, 2, 0, 3).astype(bf16))
    # w8_pack[p, et, j, k] = 64 * We[et*256 + 2p + j, k]
    w8_pack = np.ascontiguousarray(
        (We * WSCALE).reshape(ET2, 128, 2, K).transpose(1, 0, 2, 3).astype(f8))
    b_pack = np.ascontiguousarray(
        np.asarray(b_attn, dtype=np.float32).reshape(1, K).astype(bf16))
    # v_pack[p, kt] = v[kt*128 + p]
    v_pack = np.ascontiguousarray(
        np.asarray(v, dtype=np.float32).reshape(KT, 128).T.astype(bf16))
    hidden = np.asarray(hidden, dtype=np.float32)

    in_maps = []
    for c in range(NCORES):
        sl = slice(c * BL, (c + 1) * BL)
        # hT_pack[p, dt*BL + b] = hidden[b, dt*128 + p]
        hT_pack = np.ascontiguousarray(
            hidden[sl].T.reshape(KT, 128, BL).transpose(1, 0, 2)
            .reshape(128, KT * BL).astype(bf16))
        in_maps.append({
            "encoder_outputs": np.ascontiguousarray(encoder_outputs[sl]),
            "mask": np.ascontiguousarray(np.asarray(mask[sl], dtype=np.int32)),
            "wh_pack": wh_pack,
            "w8_pack": w8_pack,
            "hT_pack": hT_pack,
            "b_pack": b_pack,
            "v_pack": v_pack,
        })
    return in_maps


def kernel(hidden, encoder_outputs, mask, W_attn, b_attn, v):
    from concourse.bass_utils import run_bass_kernel_spmd

    nc = get_nc()
    in_maps = make_in_maps(hidden, encoder_outputs, mask, W_attn, b_attn, v)
    res = run_bass_kernel_spmd(nc, in_maps, core_ids=list(range(NCORES)))
    return np.concatenate(
        [np.asarray(res.results[c]["out"], dtype=np.float32) for c in range(NCORES)],
        axis=0,
    )


# revision 17
# speedup vs baseline: 1.0505x; 1.0505x over previous
"""Bahdanau-attention scores kernel for Trainium2 (8 NeuronCores, SPMD).

Computation (per batch row b):
    pre[s, k] = hidden[b] @ Wh + enc[b, s] @ We + b_attn       (S=1024, E=K=1024)
    scores[s] = tanh(pre[s, :]) @ v
    out[b]    = softmax(where(mask[b]==0, -1e10, scores))      over s

Sharding: data-parallel over batch B=64 -> 8 batches per core; weights
replicated. No collectives.

Per-core structure (fp8 DoubleRow main matmul, bf16 elsewhere):
  - enc HBM->fp8-pair pipeline runs entirely on HWDGE rings + DVE, with NO
    SWDGE DMAs in flight: Tile fences xbar transposes against every
    in-flight SWDGE DMA (both directions, all 8 DMASW queues), which in the
    previous DRAM->DRAM SWDGE-cast design serialized cast(13.4us) ->
    tp-issue(4.2) -> tp(5.2) into the 23.9us/batch critical path.  Now:
      scalar ring: enc f32 DRAM->SBUF loads (natural [p=s%128, st, e])
      DVE:         tensor_copy f32 -> fp8e4 (adjacent-e pairs land in
                   adjacent bytes = the DoubleRow rhs pairing)
      sync ring:   fp8 SBUF->DRAM bounce store, then DRAM->SBUF xbar
                   transpose of the u16 pairs: encT8h[p, et, s]
    HWDGE traffic/batch = 4MB(load)+1MB(store)+1MB(tp) = 6MB at the
    ~358 GB/s per-core cap = 16.8us ~= the PE's ~17us/batch: true ridge.
  - one DRAM bounce tensor per (batch, half) so no coarse DRAM-range
    tracking can serialize different batches' stores/transposes.
  - main MM: pre[k, s] = sum_et lhsT(w8) @ rhs(encT8h), DoubleRow, one
    LDWEIGHTS per (et, kt) serving both s-halves.
  - ScalarE: tanh(psum/64 + (hidden@Wh + b_attn)[k]) -> SBUF bf16
  - hidden@Wh (hp) is interleaved per-kt into batch 0's loop (wh loaded as
    per-kt chunks) so it doesn't sit at the PE FIFO head blocking the
    first main MMs behind a 2MB weight load.
  - v-dot: 4 col-tiled PE matmuls (tile_position=(0,32q)), lag THREE
    k-tiles behind the main MMs (ACT falls ~1.5 groups behind the PE by
    batch end; lag 2 stalled the PE 0.5-1.5us per batch).  Last 3 k-tiles
    carried into the next batch, one k-tile per MM group, all 4 quarter
    MMs before the 4 flat4 copies (interleaving MMs and copies created
    false column-range WARs = 3x850ns PE bubbles).
  - softmax in chunks: batches 0-5 during iter 7, 6-7 at the end; gathers
    flat4->scores are HWDGE SBUF->SBUF (no accum; mask offset added on
    DVE), so they don't fence against transposes either.

Sync note: this walrus build encodes at most ONE semaphore wait per
instruction; _split_multi_waits() rewrites Tile's multi-wait instructions
into NoOp(wait) chains on the same engine.
"""

import sys

if "/opt/trn_rl_repo" not in sys.path:
    sys.path.insert(0, "/opt/trn_rl_repo")

from contextlib import ExitStack

import numpy as np

B, S, E, K = 64, 1024, 1024, 1024  # E = 2*ENC_HID, K = DEC_HID
NCORES = 8
BL = B // NCORES  # batches per core
NEG = -1e10
WSCALE = 64.0     # We quantization scale into E4M3 range

ET2 = E // 256  # 4 DoubleRow e-tiles (256-deep contraction each)
KT = K // 128   # 8 k-tiles
ST = S // 128   # 8 s-tiles
NB = 512        # matmul free-dim block (one s-half)
SB = S // NB    # 2 s-halves
VLAG = 3        # v-dot lag in k-tiles behind the main MMs

_CACHE = {}


def _build_bass(strip=True):
    from concourse import bass, mybir, tile

    f32 = mybir.dt.float32
    bf16 = mybir.dt.bfloat16
    f8 = mybir.dt.float8e4
    u16 = mybir.dt.uint16
    i32 = mybir.dt.int32
    Tanh = mybir.ActivationFunctionType.Tanh
    Exp = mybir.ActivationFunctionType.Exp
    Alu = mybir.AluOpType
    Ax = mybir.AxisListType
    DR = mybir.MatmulPerfMode.DoubleRow

    nc = bass.Bass()

    enc_d = nc.declare_dram_parameter("encoder_outputs", [BL, S, E], f32, isOutput=False)
    # one bounce tensor per (batch, half): no shared-tensor dep tracking
    enc8_d = [[nc.dram_tensor(f"enc8_{b}_{h}", [NB, E], f8) for h in range(SB)]
              for b in range(BL)]
    mask_d = nc.declare_dram_parameter("mask", [BL, S], i32, isOutput=False)
    wh_d = nc.declare_dram_parameter("wh_pack", [128, KT, K], bf16, isOutput=False)
    w8_d = nc.declare_dram_parameter("w8_pack", [128, ET2, 2, K], f8, isOutput=False)
    hT_d = nc.declare_dram_parameter("hT_pack", [128, KT * BL], bf16, isOutput=False)
    b_d = nc.declare_dram_parameter("b_pack", [1, K], bf16, isOutput=False)
    v_d = nc.declare_dram_parameter("v_pack", [128, KT], bf16, isOutput=False)
    out_d = nc.declare_dram_parameter("out", [BL, S], f32, isOutput=True)

    with tile.TileContext(nc) as tc, ExitStack() as ctx:
        const = ctx.enter_context(tc.tile_pool(name="const", bufs=1))
        f32_pool = ctx.enter_context(tc.tile_pool(name="encf32", bufs=4))
        f8_pool = ctx.enter_context(tc.tile_pool(name="encf8", bufs=3))
        tp_pool = ctx.enter_context(tc.tile_pool(name="encT", bufs=6))
        tanh_pool = ctx.enter_context(tc.tile_pool(name="tanh", bufs=5))
        pre_ps = ctx.enter_context(tc.tile_pool(name="pre_ps", bufs=3, space="PSUM"))
        sc_ps = ctx.enter_context(tc.tile_pool(name="sc_ps", bufs=1, space="PSUM"))
        fin = ctx.enter_context(tc.tile_pool(name="fin", bufs=1))

        # ---- weights (host-packed) ----
        # w8 gates the first main matmul: first on the sync ring, in two
        # DMAs so they overlap (per-DMA rate is ~143 GB/s)
        w8 = const.tile([128, ET2, 2, K], f8)
        nc.sync.dma_start(w8[:, :2], w8_d[:, :2])
        nc.sync.dma_start(w8[:, 2:], w8_d[:, 2:])
        hT_bf = const.tile([128, KT * BL], bf16)
        nc.sync.dma_start(hT_bf[:], hT_d[:])
        b_attn_bf = const.tile([1, K], bf16)
        nc.sync.dma_start(b_attn_bf[:], b_d[:])
        v_bf = const.tile([128, KT], bf16)
        nc.sync.dma_start(v_bf[:], v_d[:])
        # wh per-kt chunks on the scalar ring (repacked kt-major on host)
        wh_bf = const.tile([128, KT, KT, 128], bf16)  # [p, kt, dt, c]

        ones_bf = const.tile([1, BL], bf16)
        nc.vector.memset(ones_bf[:], 1.0)

        def stage_load(b, h):
            """enc f32 half-batch -> SBUF [p, st, e], s = st*128 + p.
            Two 1MB quarter DMAs per half: a single 2MB HWDGE DMA tops out
            ~143 GB/s; the ring needs several 1MB DMAs in flight to reach
            the ~358 GB/s aggregate."""
            t = f32_pool.tile([128, ST // SB, E], f32, tag="encf32",
                              name=f"encf32_{b}_{h}")
            for q in range(2):
                nc.sync.dma_start(
                    t[:, q * 2:(q + 1) * 2],
                    enc_d[b, h * NB + q * 256:h * NB + (q + 1) * 256, :]
                    .rearrange("(st p) e -> p st e", p=128))
            return t

        def stage_cast(t32, b, h):
            t8 = f8_pool.tile([128, ST // SB, E], f8, tag="encf8",
                              name=f"encf8_{b}_{h}")
            nc.vector.tensor_copy(t8[:], t32[:])
            return t8

        def stage_store(t8, b, h):
            nc.sync.dma_start(
                enc8_d[b][h][:].rearrange("(st p) e -> p st e", p=128), t8[:])

        def stage_tp(b, h):
            """DRAM->SBUF u16-pair xbar transpose of the fp8 bounce half:
            encT8h u16[p, et, s] = fp8 pair
            (enc[b, h*512 + s, et*256+2p], enc[b, h*512 + s, et*256+2p+1]).
            Issued from the ACT engine: the transposes get their own ring
            (the sync ring is saturated with loads+stores, and an xbar
            transpose in that queue would head-of-line block it at the
            slower ~178 GB/s xbar rate)."""
            eh = tp_pool.tile([128, ET2, NB], u16, tag="encT",
                              name=f"encT_{b}_{h}")
            nc.scalar.dma_start(eh[:], enc8_d[b][h][:].bitcast(u16),
                                transpose=True)
            return eh

        def rhs_view(eh, f8dt):
            return eh[:].bitcast(f8dt).rearrange("p et (s j) -> p et j s", j=2)

        hpb = const.tile([128, KT * BL], f32)  # col = kt*BL + b

        def emit_hp_kt(kt):
            # h_proj[k, b] = sum_d Wh[d, k]*hidden[b, d] + b_attn[k],
            # one k-tile at a time, interleaved into batch 0's MM stream
            hp_ps = pre_ps.tile([128, NB], f32, tag="pre", name=f"hp_ps{kt}",
                                bufs=7)
            for dt in range(KT):
                nc.tensor.matmul(
                    hp_ps[:, :BL],
                    wh_bf[:, kt, dt, :],
                    hT_bf[:, dt * BL:(dt + 1) * BL],
                    start=(dt == 0),
                    stop=False,
                )
            nc.tensor.matmul(
                hp_ps[:, :BL],
                b_attn_bf[:, kt * 128:(kt + 1) * 128],
                ones_bf[:],
                start=False,
                stop=True,
            )
            # ACT, not DVE: the DVE is busy with next-batch casts at b0
            # start and the first tanh would block behind them (gpsimd
            # cannot read PSUM)
            nc.scalar.copy(hpb[:, kt * BL:(kt + 1) * BL], hp_ps[:, :BL])

        # scores accumulate on PSUM rows 32q (col-group q = s-quarter q);
        # staged in flat4 rows 32q on SBUF, gathered to [b, S] chunks
        flat4 = fin.tile([97, BL * 256], f32)

        # softmax state
        mask_i = fin.tile([BL, S], i32)
        maskoff = fin.tile([BL, S], f32)
        scores = fin.tile([BL, S], f32)
        scm = fin.tile([BL, S], f32)
        negmax = fin.tile([BL, 1], f32)
        expv = fin.tile([BL, S], f32)
        rowsum = fin.tile([BL, 1], f32)
        recip = fin.tile([BL, 1], f32)
        outf = fin.tile([BL, S], f32)

        def emit_mask_prep():
            nc.scalar.dma_start(mask_i[:], mask_d[:])
            nc.vector.tensor_copy(maskoff[:], mask_i[:])
            nc.vector.tensor_scalar(
                maskoff[:], maskoff[:], -NEG, NEG, Alu.mult, Alu.add)

        def emit_softmax_chunk(b0, b1):
            # gather flat4 quarter-rows into [b, s] layout (HWDGE SBUF->SBUF)
            for q in range(4):
                nc.scalar.dma_start(
                    scores[b0:b1, q * 256:(q + 1) * 256],
                    flat4[32 * q:32 * q + 1, b0 * 256:b1 * 256])
            # compute ops always span [0, b1): DVE/ACT partition windows
            # must start at 0 (quadrant alignment); recomputing the already
            # finished low batches is idempotent and partition-parallel
            nc.vector.tensor_add(
                scm[:b1, :], scores[:b1, :], maskoff[:b1, :])
            nc.vector.tensor_reduce(
                negmax[:b1], scm[:b1, :], Ax.X, Alu.max, negate=True)
            nc.scalar.activation(
                expv[:b1, :], scm[:b1, :], Exp, bias=negmax[:b1],
                accum_out=rowsum[:b1])
            nc.vector.reciprocal(recip[:b1], rowsum[:b1])
            nc.vector.tensor_scalar_mul(
                outf[:b1, :], expv[:b1, :], recip[:b1])
            nc.scalar.dma_start(out_d[b0:b1, :], outf[b0:b1, :])

        # ---- prologue ----
        t32 = {}
        t32[(0, 0)] = stage_load(0, 0)
        t32[(0, 1)] = stage_load(0, 1)
        # wh chunks on the scalar ring (free until the first transpose)
        for ck in range(KT):
            # wh_d dims are [p, kt, (dt c)] after the host kt-major repack
            nc.scalar.dma_start(wh_bf[:, ck], wh_d[:, ck])
        encTs = {}

        def stage_cs(b):
            # DVE casts + sync-ring stores; the transposes are emitted
            # separately mid-MM-loop (stage_tps) so their store-sem waits
            # never block the ACT engine's tanh stream at iteration top
            t8s = [stage_cast(t32.pop((b, h)), b, h) for h in range(SB)]
            for h in range(SB):
                stage_store(t8s[h], b, h)

        def stage_tps(b):
            for h in range(SB):
                encTs[(b, h)] = stage_tp(b, h)

        stage_cs(0)
        # b1 loads go on the sync ring after b0's stores
        t32[(1, 0)] = stage_load(1, 0)
        t32[(1, 1)] = stage_load(1, 1)
        stage_tps(0)

        # scores PSUM: ONE bank, halves alternated by batch parity
        scband = sc_ps.tile([128, 2, 256], f32, tag="sc", name="scband")

        def scq(b, q):
            return scband[32 * q:32 * q + 1, b % 2, :]

        carry = []  # [(b_prev, kt, th_tile, col_off, qs)] not yet emitted

        def emit_vdots(b, kt, th, col_off, qs, stop):
            for q in qs:
                nc.tensor.matmul(
                    scq(b, q),
                    v_bf[:, kt:kt + 1],
                    th[:, col_off + (q - qs[0]) * 256:
                       col_off + (q - qs[0] + 1) * 256],
                    start=(kt == 0), stop=stop,
                    tile_position=(0, 32 * q))
            if stop:
                # all MMs above before any copy: interleaving creates false
                # column-range WARs that bubble the PE ~850ns per quarter
                for q in qs:
                    nc.vector.tensor_copy(
                        flat4[32 * q:32 * q + 1, b * 256:(b + 1) * 256],
                        scq(b, q))

        def emit_carry(budget):
            while carry and budget > 0:
                b_p, kt, th, col_off, qs = carry.pop(0)
                emit_vdots(b_p, kt, th, col_off, qs, stop=(kt == KT - 1))
                budget -= 1

        # ---- main loop over local batches (software-pipelined) ----
        for b in range(BL):
            # stage next batches: casts+stores(b+1) first (their sync-ring
            # DMAs must precede the 4MB of loads(b+2) in the queue so the
            # transposes can start mid-iteration)
            if b + 1 < BL:
                stage_cs(b + 1)
            if b + 2 < BL:
                t32[(b + 2, 0)] = stage_load(b + 2, 0)
                t32[(b + 2, 1)] = stage_load(b + 2, 1)
            if b == 1:
                emit_mask_prep()

            if b == 0:
                # sb-major: start on the first transposed s-half immediately;
                # hp(kt) interleaved into the sb0 pass
                ths = {}
                for sb in range(SB):
                    rh = rhs_view(encTs.pop((0, sb)), f8)
                    for kt in range(KT):
                        pre = pre_ps.tile([128, NB], f32, tag="pre",
                                          name="preh", bufs=7)
                        for et in range(ET2):
                            nc.tensor.matmul(
                                pre[:],
                                w8[:, et, :, kt * 128:(kt + 1) * 128],
                                rh[:, et, :, :],
                                start=(et == 0),
                                stop=(et == ET2 - 1),
                                perf_mode=DR,
                            )
                        if sb == 0:
                            emit_hp_kt(kt)
                            if kt == 6:
                                stage_tps(1)
                        th = tanh_pool.tile([128, NB], bf16, tag="thh",
                                            name="thh", bufs=5)
                        nc.scalar.activation(
                            th[:], pre[:], Tanh,
                            bias=hpb[:, kt * BL:kt * BL + 1],
                            scale=1.0 / WSCALE,
                        )
                        ths[(sb, kt)] = th
                        if kt >= 1:
                            emit_vdots(0, kt - 1, ths[(sb, kt - 1)], 0,
                                       (2 * sb, 2 * sb + 1), stop=False)
                    carry.append((0, KT - 1, ths[(sb, KT - 1)], 0,
                                  (2 * sb, 2 * sb + 1)))
            else:
                rhA = rhs_view(encTs.pop((b, 0)), f8)
                rhB = rhs_view(encTs.pop((b, 1)), f8)
                ths = {}
                for kt in range(KT):
                    pres = [pre_ps.tile([128, NB], f32, tag="pre",
                                        name=f"pre{sb}", bufs=7)
                            for sb in range(SB)]
                    for et in range(ET2):  # one LDWEIGHTS serves both sb
                        for sb, rh in ((0, rhA), (1, rhB)):
                            nc.tensor.matmul(
                                pres[sb][:],
                                w8[:, et, :, kt * 128:(kt + 1) * 128],
                                rh[:, et, :, :],
                                start=(et == 0),
                                stop=(et == ET2 - 1),
                                perf_mode=DR,
                            )
                    if kt < VLAG:
                        emit_carry(1)
                    if kt == 4 and b + 1 < BL:
                        stage_tps(b + 1)
                    th = tanh_pool.tile([128, SB * NB], bf16, tag="tanh",
                                        bufs=5)
                    for sb in range(SB):
                        nc.scalar.activation(
                            th[:, sb * NB:(sb + 1) * NB], pres[sb][:], Tanh,
                            bias=hpb[:, kt * BL + b:kt * BL + b + 1],
                            scale=1.0 / WSCALE,
                        )
                    ths[kt] = th
                    if kt >= VLAG:
                        emit_vdots(b, kt - VLAG, ths[kt - VLAG], 0,
                                   (0, 1, 2, 3), stop=False)
                for kt in range(KT - VLAG, KT):
                    carry.append((b, kt, ths[kt], 0, (0, 1, 2, 3)))
            if b == 7:
                emit_softmax_chunk(0, 6)

        emit_carry(len(carry))
        emit_softmax_chunk(6, 8)

    if strip:
        _split_multi_waits(nc, mybir)
    return nc


def _split_multi_waits(nc, mybir):
    """Move extra semaphore waits onto standalone NoOps on the same engine.

    This walrus build encodes at most one sync-wait command per instruction,
    but Tile emits instructions with several (cross-engine RAW + WAR + DMA
    queue ordering). A NoOp carrying one wait, placed immediately before the
    instruction in the same engine's stream, is semantically identical: the
    engine's sequencer blocks on the NoOp's wait before dispatching the real
    instruction.
    """
    n = 0
    for fn in nc.m.functions:
        for blk in fn.blocks:
            insts = blk.instructions
            new = []
            changed = False
            for inst in insts:
                si = inst.sync_info
                if si is not None and si.on_wait and len(si.on_wait) > 1:
                    for w in list(si.on_wait)[:-1]:
                        n += 1
                        new.append(mybir.InstNoOp(
                            name=f"{inst.name}-sw{n}",
                            engine=inst.engine,
                            text_hint="split_wait",
                            bass_nofuse=True,
                            sync_info=mybir.SyncInfo(
                                on_wait=[w], on_update=[]),
                        ))
                    inst.sync_info = mybir.SyncInfo(
                        on_wait=[list(si.on_wait)[-1]],
                        on_update=list(si.on_update or []))
                    changed = True
                new.append(inst)
            if changed:
                blk.instructions = new


def get_nc(strip=True):
    key = ("nc", strip)
    if key not in _CACHE:
        _CACHE[key] = _build_bass(strip)
    return _CACHE[key]


def make_in_maps(hidden, encoder_outputs, mask, W_attn, b_attn, v):
    import ml_dtypes

    bf16 = ml_dtypes.bfloat16
    f8 = ml_dtypes.float8_e4m3

    W_attn = np.asarray(W_attn, dtype=np.float32)
    Wh, We = W_attn[:K], W_attn[K:]
    # wh_pack[p, kt, dt, c] = Wh[dt*128 + p, kt*128 + c]  (kt-major chunks)
    wh_pack = np.ascontiguousarray(
        Wh.reshape(KT, 128, KT, 128).transpose(1, 2, 0, 3).astype(bf16))
    # w8_pack[p, et, j, k] = 64 * We[et*256 + 2p + j, k]
    w8_pack = np.ascontiguousarray(
        (We * WSCALE).reshape(ET2, 128, 2, K).transpose(1, 0, 2, 3).astype(f8))
    b_pack = np.ascontiguousarray(
        np.asarray(b_attn, dtype=np.float32).reshape(1, K).astype(bf16))
    # v_pack[p, kt] = v[kt*128 + p]
    v_pack = np.ascontiguousarray(
        np.asarray(v, dtype=np.float32).reshape(KT, 128).T.astype(bf16))
    hidden = np.asarray(hidden, dtype=np.float32)

    in_maps = []
    for c in range(NCORES):
        sl = slice(c * BL, (c + 1) * BL)
        # hT_pack[p, dt*BL + b] = hidden[b, dt*128 + p]
        hT_pack = np.ascontiguousarray(
            hidden[sl].T.reshape(KT, 128, BL).transpose(1, 0, 2)
            .reshape(128, KT * BL).astype(bf16))
        in_maps.append({
            "encoder_outputs": np.ascontiguousarray(encoder_outputs[sl]),
            "mask": np.ascontiguousarray(np.asarray(mask[sl], dtype=np.int32)),
            "wh_pack": wh_pack,
            "w8_pack": w8_pack,
            "hT_pack": hT_pack,
            "b_pack": b_pack,
            "v_pack": v_pack,
        })
    return in_maps


def kernel(hidden, encoder_outputs, mask, W_attn, b_attn, v):
    from concourse.bass_utils import run_bass_kernel_spmd

    nc = get_nc()
    in_maps = make_in_maps(hidden, encoder_outputs, mask, W_attn, b_attn, v)
    res = run_bass_kernel_spmd(nc, in_maps, core_ids=list(range(NCORES)))
    return np.concatenate(
        [np.asarray(res.results[c]["out"], dtype=np.float32) for c in range(NCORES)],
        axis=0,
    )


# revision 23
# speedup vs baseline: 1.0571x; 1.0063x over previous
"""Bahdanau-attention scores kernel for Trainium2 (8 NeuronCores, SPMD).

Computation (per batch row b):
    pre[s, k] = hidden[b] @ Wh + enc[b, s] @ We + b_attn       (S=1024, E=K=1024)
    scores[s] = tanh(pre[s, :]) @ v
    out[b]    = softmax(where(mask[b]==0, -1e10, scores))      over s

Sharding: data-parallel over batch B=64 -> 8 batches per core; weights
replicated. No collectives.

Per-core structure (fp8 DoubleRow main matmul, bf16 elsewhere):
  - enc HBM->fp8-pair pipeline runs entirely on HWDGE rings + DVE, with NO
    SWDGE DMAs in flight: Tile fences xbar transposes against every
    in-flight SWDGE DMA (both directions, all 8 DMASW queues), which in the
    previous DRAM->DRAM SWDGE-cast design serialized cast(13.4us) ->
    tp-issue(4.2) -> tp(5.2) into the 23.9us/batch critical path.  Now:
      scalar ring: enc f32 DRAM->SBUF loads (natural [p=s%128, st, e])
      DVE:         tensor_copy f32 -> fp8e4 (adjacent-e pairs land in
                   adjacent bytes = the DoubleRow rhs pairing)
      sync ring:   fp8 SBUF->DRAM bounce store, then DRAM->SBUF xbar
                   transpose of the u16 pairs: encT8h[p, et, s]
    HWDGE traffic/batch = 4MB(load)+1MB(store)+1MB(tp) = 6MB at the
    ~358 GB/s per-core cap = 16.8us ~= the PE's ~17us/batch: true ridge.
  - one DRAM bounce tensor per (batch, half) so no coarse DRAM-range
    tracking can serialize different batches' stores/transposes.
  - main MM: pre[k, s] = sum_et lhsT(w8) @ rhs(encT8h), DoubleRow, one
    LDWEIGHTS per (et, kt) serving both s-halves.
  - ScalarE: tanh(psum/64 + (hidden@Wh + b_attn)[k]) -> SBUF bf16
  - hidden@Wh (hp) is interleaved per-kt into batch 0's loop (wh loaded as
    per-kt chunks) so it doesn't sit at the PE FIFO head blocking the
    first main MMs behind a 2MB weight load.
  - v-dot: 4 col-tiled PE matmuls (tile_position=(0,32q)), lag THREE
    k-tiles behind the main MMs (ACT falls ~1.5 groups behind the PE by
    batch end; lag 2 stalled the PE 0.5-1.5us per batch).  Last 3 k-tiles
    carried into the next batch, one k-tile per MM group, all 4 quarter
    MMs before the 4 flat4 copies (interleaving MMs and copies created
    false column-range WARs = 3x850ns PE bubbles).
  - softmax in chunks: batches 0-5 during iter 7, 6-7 at the end; gathers
    flat4->scores are HWDGE SBUF->SBUF (no accum; mask offset added on
    DVE), so they don't fence against transposes either.

Sync note: this walrus build encodes at most ONE semaphore wait per
instruction; _split_multi_waits() rewrites Tile's multi-wait instructions
into NoOp(wait) chains on the same engine.
"""

import sys

if "/opt/trn_rl_repo" not in sys.path:
    sys.path.insert(0, "/opt/trn_rl_repo")

from contextlib import ExitStack

import numpy as np

B, S, E, K = 64, 1024, 1024, 1024  # E = 2*ENC_HID, K = DEC_HID
NCORES = 8
BL = B // NCORES  # batches per core
NEG = -1e10
WSCALE = 64.0     # We quantization scale into E4M3 range

ET2 = E // 256  # 4 DoubleRow e-tiles (256-deep contraction each)
KT = K // 128   # 8 k-tiles
ST = S // 128   # 8 s-tiles
NB = 512        # matmul free-dim block (one s-half)
SB = S // NB    # 2 s-halves
VLAG = 3        # v-dot lag in k-tiles behind the main MMs

_CACHE = {}


def _build_bass(strip=True):
    from concourse import bass, mybir, tile

    f32 = mybir.dt.float32
    bf16 = mybir.dt.bfloat16
    f8 = mybir.dt.float8e4
    u16 = mybir.dt.uint16
    i32 = mybir.dt.int32
    Tanh = mybir.ActivationFunctionType.Tanh
    Exp = mybir.ActivationFunctionType.Exp
    Alu = mybir.AluOpType
    Ax = mybir.AxisListType
    DR = mybir.MatmulPerfMode.DoubleRow

    nc = bass.Bass()

    enc_d = nc.declare_dram_parameter("encoder_outputs", [BL, S, E], f32, isOutput=False)
    # one bounce tensor per (batch, half): no shared-tensor dep tracking
    enc8_d = [[nc.dram_tensor(f"enc8_{b}_{h}", [NB, E], f8) for h in range(SB)]
              for b in range(BL)]
    mask_d = nc.declare_dram_parameter("mask", [BL, S], i32, isOutput=False)
    wh_d = nc.declare_dram_parameter("wh_pack", [128, KT, K], bf16, isOutput=False)
    w8_d = nc.declare_dram_parameter("w8_pack", [128, ET2, 2, K], f8, isOutput=False)
    hT_d = nc.declare_dram_parameter("hT_pack", [128, KT * BL], bf16, isOutput=False)
    b_d = nc.declare_dram_parameter("b_pack", [1, K], bf16, isOutput=False)
    v_d = nc.declare_dram_parameter("v_pack", [128, KT], bf16, isOutput=False)
    out_d = nc.declare_dram_parameter("out", [BL, S], f32, isOutput=True)

    with tile.TileContext(nc) as tc, ExitStack() as ctx:
        const = ctx.enter_context(tc.tile_pool(name="const", bufs=1))
        tp_pool = ctx.enter_context(tc.tile_pool(name="encT", bufs=6))
        tanh_pool = ctx.enter_context(tc.tile_pool(name="tanh", bufs=5))
        pre_ps = ctx.enter_context(tc.tile_pool(name="pre_ps", bufs=3, space="PSUM"))
        sc_ps = ctx.enter_context(tc.tile_pool(name="sc_ps", bufs=1, space="PSUM"))
        fin = ctx.enter_context(tc.tile_pool(name="fin", bufs=1))

        # ---- weights (host-packed) ----
        # w8 gates the first main matmul: first on the sync ring, in two
        # DMAs so they overlap (per-DMA rate is ~143 GB/s)
        w8 = const.tile([128, ET2, 2, K], f8)
        nc.sync.dma_start(w8[:, :2], w8_d[:, :2])
        nc.sync.dma_start(w8[:, 2:], w8_d[:, 2:])
        hT_bf = const.tile([128, KT * BL], bf16)
        nc.sync.dma_start(hT_bf[:], hT_d[:])
        b_attn_bf = const.tile([1, K], bf16)
        nc.sync.dma_start(b_attn_bf[:], b_d[:])
        v_bf = const.tile([128, KT], bf16)
        nc.sync.dma_start(v_bf[:], v_d[:])
        # wh per-kt chunks on the scalar ring (repacked kt-major on host)
        wh_bf = const.tile([128, KT, KT, 128], bf16)  # [p, kt, dt, c]

        ones_bf = const.tile([1, BL], bf16)
        nc.vector.memset(ones_bf[:], 1.0)

        def stage_cast(b, h):
            """f32 -> fp8 DRAM->DRAM SWDGE cast of one s-half, in 4 chunks
            round-robined over the SWDGE queues (the CCE conversion rate is
            the limiter, ~75 G elem/s aggregate; the 1MB fp8 write hides
            inside the 4MB read)."""
            for st in range(ST // SB):
                nc.gpsimd.dma_start(
                    enc8_d[b][h][st * 128:(st + 1) * 128, :],
                    enc_d[b, h * NB + st * 128:h * NB + (st + 1) * 128, :])

        def stage_tp(b, h):
            """DRAM->SBUF u16-pair xbar transpose of the fp8 bounce half:
            encT8h u16[p, et, s] = fp8 pair
            (enc[b, h*512 + s, et*256+2p], enc[b, h*512 + s, et*256+2p+1]).
            Every transpose is a full DMA-system barrier (Tile fences it
            against ALL in-flight DMAs, both directions), so the schedule
            strictly alternates cast(batch) / tp(batch)."""
            eh = tp_pool.tile([128, ET2, NB], u16, tag="encT",
                              name=f"encT_{b}_{h}")
            nc.sync.dma_start(eh[:], enc8_d[b][h][:].bitcast(u16),
                              transpose=True)
            return eh

        def rhs_view(eh, f8dt):
            return eh[:].bitcast(f8dt).rearrange("p et (s j) -> p et j s", j=2)

        hpb = const.tile([128, KT * BL], f32)  # col = kt*BL + b

        def emit_hp_kt(kt):
            # h_proj[k, b] = sum_d Wh[d, k]*hidden[b, d] + b_attn[k],
            # one k-tile at a time, interleaved into batch 0's MM stream
            hp_ps = pre_ps.tile([128, NB], f32, tag="pre", name=f"hp_ps{kt}",
                                bufs=7)
            for dt in range(KT):
                nc.tensor.matmul(
                    hp_ps[:, :BL],
                    wh_bf[:, kt, dt, :],
                    hT_bf[:, dt * BL:(dt + 1) * BL],
                    start=(dt == 0),
                    stop=False,
                )
            nc.tensor.matmul(
                hp_ps[:, :BL],
                b_attn_bf[:, kt * 128:(kt + 1) * 128],
                ones_bf[:],
                start=False,
                stop=True,
            )
            # ACT, not DVE: the DVE is busy with next-batch casts at b0
            # start and the first tanh would block behind them (gpsimd
            # cannot read PSUM)
            nc.scalar.copy(hpb[:, kt * BL:(kt + 1) * BL], hp_ps[:, :BL])

        # scores accumulate on PSUM rows 32q (col-group q = s-quarter q);
        # staged in flat4 rows 32q on SBUF, gathered to [b, S] chunks
        flat4 = fin.tile([97, BL * 256], f32)

        # softmax state
        mask_i = fin.tile([BL, S], i32)
        maskoff = fin.tile([BL, S], f32)
        scores = fin.tile([BL, S], f32)
        scm = fin.tile([BL, S], f32)
        negmax = fin.tile([BL, 1], f32)
        expv = fin.tile([BL, S], f32)
        rowsum = fin.tile([BL, 1], f32)
        recip = fin.tile([BL, 1], f32)
        outf = fin.tile([BL, S], f32)

        def emit_mask_prep():
            nc.scalar.dma_start(mask_i[:], mask_d[:])
            nc.vector.tensor_copy(maskoff[:], mask_i[:])
            nc.vector.tensor_scalar(
                maskoff[:], maskoff[:], -NEG, NEG, Alu.mult, Alu.add)

        def emit_softmax_chunk(b0, b1):
            # gather flat4 quarter-rows into [b, s] layout (HWDGE SBUF->SBUF)
            for q in range(4):
                nc.scalar.dma_start(
                    scores[b0:b1, q * 256:(q + 1) * 256],
                    flat4[32 * q:32 * q + 1, b0 * 256:b1 * 256])
            # compute ops always span [0, b1): DVE/ACT partition windows
            # must start at 0 (quadrant alignment); recomputing the already
            # finished low batches is idempotent and partition-parallel
            nc.vector.tensor_add(
                scm[:b1, :], scores[:b1, :], maskoff[:b1, :])
            nc.vector.tensor_reduce(
                negmax[:b1], scm[:b1, :], Ax.X, Alu.max, negate=True)
            nc.scalar.activation(
                expv[:b1, :], scm[:b1, :], Exp, bias=negmax[:b1],
                accum_out=rowsum[:b1])
            nc.vector.reciprocal(recip[:b1], rowsum[:b1])
            nc.vector.tensor_scalar_mul(
                outf[:b1, :], expv[:b1, :], recip[:b1])
            nc.scalar.dma_start(out_d[b0:b1, :], outf[b0:b1, :])

        # ---- prologue ----
        # wh chunks on the scalar ring (free until the gathers at the end)
        for ck in range(KT):
            # wh_d dims are [p, kt, (dt c)] after the host kt-major repack
            nc.scalar.dma_start(wh_bf[:, ck], wh_d[:, ck])
        encTs = {}

        def stage_tps(b):
            for h in range(SB):
                encTs[(b, h)] = stage_tp(b, h)

        # strict cast/tp alternation from the start: bulk-casting ahead
        # would only make the first transpose's fence wait for all of it
        stage_cast(0, 0)
        encTs[(0, 0)] = stage_tp(0, 0)
        stage_cast(0, 1)
        encTs[(0, 1)] = stage_tp(0, 1)
        stage_cast(1, 0)
        stage_cast(1, 1)

        # scores PSUM: ONE bank, halves alternated by batch parity
        scband = sc_ps.tile([128, 2, 256], f32, tag="sc", name="scband")

        def scq(b, q):
            return scband[32 * q:32 * q + 1, b % 2, :]

        carry = []  # [(b_prev, kt, th_tile, col_off, qs)] not yet emitted

        def emit_vdots(b, kt, th, col_off, qs, stop):
            for q in qs:
                nc.tensor.matmul(
                    scq(b, q),
                    v_bf[:, kt:kt + 1],
                    th[:, col_off + (q - qs[0]) * 256:
                       col_off + (q - qs[0] + 1) * 256],
                    start=(kt == 0), stop=stop,
                    tile_position=(0, 32 * q))
            if stop:
                # all MMs above before any copy: interleaving creates false
                # column-range WARs that bubble the PE ~850ns per quarter
                for q in qs:
                    nc.vector.tensor_copy(
                        flat4[32 * q:32 * q + 1, b * 256:(b + 1) * 256],
                        scq(b, q))

        def emit_carry(budget):
            while carry and budget > 0:
                b_p, kt, th, col_off, qs = carry.pop(0)
                emit_vdots(b_p, kt, th, col_off, qs, stop=(kt == KT - 1))
                budget -= 1

        # ---- main loop over local batches (software-pipelined) ----
        for b in range(BL):
            # tps(b+1) FIRST: they only wait cast(b+1) (done last iter), so
            # encT(b+1) lands ~14us before it's needed; casts(b+2) then run
            # behind the tp fence for the rest of the iteration
            if b + 1 < BL:
                stage_tps(b + 1)
            if b + 2 < BL:
                stage_cast(b + 2, 0)
                stage_cast(b + 2, 1)
            if b == 1:
                emit_mask_prep()

            if b == 0:
                # sb-major: start on the first transposed s-half immediately;
                # hp(kt) interleaved into the sb0 pass
                ths = {}
                for sb in range(SB):
                    rh = rhs_view(encTs.pop((0, sb)), f8)
                    for kt in range(KT):
                        pre = pre_ps.tile([128, NB], f32, tag="pre",
                                          name="preh", bufs=7)
                        for et in range(ET2):
                            nc.tensor.matmul(
                                pre[:],
                                w8[:, et, :, kt * 128:(kt + 1) * 128],
                                rh[:, et, :, :],
                                start=(et == 0),
                                stop=(et == ET2 - 1),
                                perf_mode=DR,
                            )
                        if sb == 0:
                            emit_hp_kt(kt)
                        th = tanh_pool.tile([128, NB], bf16, tag="thh",
                                            name="thh", bufs=5)
                        nc.scalar.activation(
                            th[:], pre[:], Tanh,
                            bias=hpb[:, kt * BL:kt * BL + 1],
                            scale=1.0 / WSCALE,
                        )
                        ths[(sb, kt)] = th
                        if kt >= 1:
                            emit_vdots(0, kt - 1, ths[(sb, kt - 1)], 0,
                                       (2 * sb, 2 * sb + 1), stop=False)
                    carry.append((0, KT - 1, ths[(sb, KT - 1)], 0,
                                  (2 * sb, 2 * sb + 1)))
            else:
                rhA = rhs_view(encTs.pop((b, 0)), f8)
                rhB = rhs_view(encTs.pop((b, 1)), f8)
                ths = {}
                for kt in range(KT):
                    pres = [pre_ps.tile([128, NB], f32, tag="pre",
                                        name=f"pre{sb}", bufs=7)
                            for sb in range(SB)]
                    for et in range(ET2):  # one LDWEIGHTS serves both sb
                        for sb, rh in ((0, rhA), (1, rhB)):
                            nc.tensor.matmul(
                                pres[sb][:],
                                w8[:, et, :, kt * 128:(kt + 1) * 128],
                                rh[:, et, :, :],
                                start=(et == 0),
                                stop=(et == ET2 - 1),
                                perf_mode=DR,
                            )
                    if kt < VLAG:
                        emit_carry(1)
                    th = tanh_pool.tile([128, SB * NB], bf16, tag="tanh",
                                        bufs=5)
                    for sb in range(SB):
                        nc.scalar.activation(
                            th[:, sb * NB:(sb + 1) * NB], pres[sb][:], Tanh,
                            bias=hpb[:, kt * BL + b:kt * BL + b + 1],
                            scale=1.0 / WSCALE,
                        )
                    ths[kt] = th
                    if kt >= VLAG:
                        emit_vdots(b, kt - VLAG, ths[kt - VLAG], 0,
                                   (0, 1, 2, 3), stop=False)
                for kt in range(KT - VLAG, KT):
                    carry.append((b, kt, ths[kt], 0, (0, 1, 2, 3)))
            if b == 7:
                emit_softmax_chunk(0, 6)

        emit_carry(len(carry))
        emit_softmax_chunk(6, 8)

    if strip:
        _split_multi_waits(nc, mybir)
    return nc


def _split_multi_waits(nc, mybir):
    """Move extra semaphore waits onto standalone NoOps on the same engine.

    This walrus build encodes at most one sync-wait command per instruction,
    but Tile emits instructions with several (cross-engine RAW + WAR + DMA
    queue ordering). A NoOp carrying one wait, placed immediately before the
    instruction in the same engine's stream, is semantically identical: the
    engine's sequencer blocks on the NoOp's wait before dispatching the real
    instruction.
    """
    n = 0
    for fn in nc.m.functions:
        for blk in fn.blocks:
            insts = blk.instructions
            new = []
            changed = False
            for inst in insts:
                si = inst.sync_info
                if si is not None and si.on_wait and len(si.on_wait) > 1:
                    for w in list(si.on_wait)[:-1]:
                        n += 1
                        new.append(mybir.InstNoOp(
                            name=f"{inst.name}-sw{n}",
                            engine=inst.engine,
                            text_hint="split_wait",
                            bass_nofuse=True,
                            sync_info=mybir.SyncInfo(
                                on_wait=[w], on_update=[]),
                        ))
                    inst.sync_info = mybir.SyncInfo(
                        on_wait=[list(si.on_wait)[-1]],
                        on_update=list(si.on_update or []))
                    changed = True
                new.append(inst)
            if changed:
                blk.instructions = new


def get_nc(strip=True):
    key = ("nc", strip)
    if key not in _CACHE:
        _CACHE[key] = _build_bass(strip)
    return _CACHE[key]


def make_in_maps(hidden, encoder_outputs, mask, W_attn, b_attn, v):
    import ml_dtypes

    bf16 = ml_dtypes.bfloat16
    f8 = ml_dtypes.float8_e4m3

    W_attn = np.asarray(W_attn, dtype=np.float32)
    Wh, We = W_attn[:K], W_attn[K:]
    # wh_pack[p, kt, dt, c] = Wh[dt*128 + p, kt*128 + c]  (kt-major chunks)
    wh_pack = np.ascontiguousarray(
        Wh.reshape(KT, 128, KT, 128).transpose(1, 2, 0, 3).astype(bf16))
    # w8_pack[p, et, j, k] = 64 * We[et*256 + 2p + j, k]
    w8_pack = np.ascontiguousarray(
        (We * WSCALE).reshape(ET2, 128, 2, K).transpose(1, 0, 2, 3).astype(f8))
    b_pack = np.ascontiguousarray(
        np.asarray(b_attn, dtype=np.float32).reshape(1, K).astype(bf16))
    # v_pack[p, kt] = v[kt*128 + p]
    v_pack = np.ascontiguousarray(
        np.asarray(v, dtype=np.float32).reshape(KT, 128).T.astype(bf16))
    hidden = np.asarray(hidden, dtype=np.float32)

    in_maps = []
    for c in range(NCORES):
        sl = slice(c * BL, (c + 1) * BL)
        # hT_pack[p, dt*BL + b] = hidden[b, dt*128 + p]
        hT_pack = np.ascontiguousarray(
            hidden[sl].T.reshape(KT, 128, BL).transpose(1, 0, 2)
            .reshape(128, KT * BL).astype(bf16))
        in_maps.append({
            "encoder_outputs": np.ascontiguousarray(encoder_outputs[sl]),
            "mask": np.ascontiguousarray(np.asarray(mask[sl], dtype=np.int32)),
            "wh_pack": wh_pack,
            "w8_pack": w8_pack,
            "hT_pack": hT_pack,
            "b_pack": b_pack,
            "v_pack": v_pack,
        })
    return in_maps


def kernel(hidden, encoder_outputs, mask, W_attn, b_attn, v):
    from concourse.bass_utils import run_bass_kernel_spmd

    nc = get_nc()
    in_maps = make_in_maps(hidden, encoder_outputs, mask, W_attn, b_attn, v)
    res = run_bass_kernel_spmd(nc, in_maps, core_ids=list(range(NCORES)))
    return np.concatenate(
        [np.asarray(res.results[c]["out"], dtype=np.float32) for c in range(NCORES)],
        axis=0,
    )


# revision 25
# speedup vs baseline: 1.1044x; 1.0448x over previous
"""Bahdanau-attention scores kernel for Trainium2 (8 NeuronCores, SPMD).

Computation (per batch row b):
    pre[s, k] = hidden[b] @ Wh + enc[b, s] @ We + b_attn       (S=1024, E=K=1024)
    scores[s] = tanh(pre[s, :]) @ v
    out[b]    = softmax(where(mask[b]==0, -1e10, scores))      over s

Sharding: data-parallel over batch B=64 -> 8 batches per core; weights
replicated. No collectives.

Per-core structure (fp8 DoubleRow main matmul, bf16 elsewhere):
  - enc HBM->fp8-pair pipeline runs entirely on HWDGE rings + DVE, with NO
    SWDGE DMAs in flight: Tile fences xbar transposes against every
    in-flight SWDGE DMA (both directions, all 8 DMASW queues), which in the
    previous DRAM->DRAM SWDGE-cast design serialized cast(13.4us) ->
    tp-issue(4.2) -> tp(5.2) into the 23.9us/batch critical path.  Now:
      scalar ring: enc f32 DRAM->SBUF loads (natural [p=s%128, st, e])
      DVE:         tensor_copy f32 -> fp8e4 (adjacent-e pairs land in
                   adjacent bytes = the DoubleRow rhs pairing)
      sync ring:   fp8 SBUF->DRAM bounce store, then DRAM->SBUF xbar
                   transpose of the u16 pairs: encT8h[p, et, s]
    HWDGE traffic/batch = 4MB(load)+1MB(store)+1MB(tp) = 6MB at the
    ~358 GB/s per-core cap = 16.8us ~= the PE's ~17us/batch: true ridge.
  - one DRAM bounce tensor per (batch, half) so no coarse DRAM-range
    tracking can serialize different batches' stores/transposes.
  - main MM: pre[k, s] = sum_et lhsT(w8) @ rhs(encT8h), DoubleRow, one
    LDWEIGHTS per (et, kt) serving both s-halves.
  - ScalarE: tanh(psum/64 + (hidden@Wh + b_attn)[k]) -> SBUF bf16
  - hidden@Wh (hp) is interleaved per-kt into batch 0's loop (wh loaded as
    per-kt chunks) so it doesn't sit at the PE FIFO head blocking the
    first main MMs behind a 2MB weight load.
  - v-dot: 4 col-tiled PE matmuls (tile_position=(0,32q)), lag THREE
    k-tiles behind the main MMs (ACT falls ~1.5 groups behind the PE by
    batch end; lag 2 stalled the PE 0.5-1.5us per batch).  Last 3 k-tiles
    carried into the next batch, one k-tile per MM group, all 4 quarter
    MMs before the 4 flat4 copies (interleaving MMs and copies created
    false column-range WARs = 3x850ns PE bubbles).
  - softmax in chunks: batches 0-5 during iter 7, 6-7 at the end; gathers
    flat4->scores are HWDGE SBUF->SBUF (no accum; mask offset added on
    DVE), so they don't fence against transposes either.

Sync note: this walrus build encodes at most ONE semaphore wait per
instruction; _split_multi_waits() rewrites Tile's multi-wait instructions
into NoOp(wait) chains on the same engine.
"""

import sys

if "/opt/trn_rl_repo" not in sys.path:
    sys.path.insert(0, "/opt/trn_rl_repo")

from contextlib import ExitStack

import numpy as np

B, S, E, K = 64, 1024, 1024, 1024  # E = 2*ENC_HID, K = DEC_HID
NCORES = 8
BL = B // NCORES  # batches per core
NEG = -1e10
WSCALE = 64.0     # We quantization scale into E4M3 range

ET2 = E // 256  # 4 DoubleRow e-tiles (256-deep contraction each)
KT = K // 128   # 8 k-tiles
ST = S // 128   # 8 s-tiles
NB = 512        # matmul free-dim block (one s-half)
SB = S // NB    # 2 s-halves
VLAG = 3        # v-dot lag in k-tiles behind the main MMs

_CACHE = {}


def _build_bass(strip=True):
    from concourse import bass, mybir, tile

    f32 = mybir.dt.float32
    bf16 = mybir.dt.bfloat16
    f8 = mybir.dt.float8e4
    u16 = mybir.dt.uint16
    i32 = mybir.dt.int32
    Tanh = mybir.ActivationFunctionType.Tanh
    Exp = mybir.ActivationFunctionType.Exp
    Alu = mybir.AluOpType
    Ax = mybir.AxisListType
    DR = mybir.MatmulPerfMode.DoubleRow

    nc = bass.Bass()

    enc_d = nc.declare_dram_parameter("encoder_outputs", [BL, S, E], f32, isOutput=False)
    # one bounce tensor per (batch, half): no shared-tensor dep tracking
    enc8_d = [[nc.dram_tensor(f"enc8_{b}_{h}", [NB, E], f8) for h in range(SB)]
              for b in range(BL)]
    mask_d = nc.declare_dram_parameter("mask", [BL, S], i32, isOutput=False)
    wh_d = nc.declare_dram_parameter("wh_pack", [128, KT, K], bf16, isOutput=False)
    w8_d = nc.declare_dram_parameter("w8_pack", [128, ET2, 2, K], f8, isOutput=False)
    hT_d = nc.declare_dram_parameter("hT_pack", [128, KT * BL], bf16, isOutput=False)
    b_d = nc.declare_dram_parameter("b_pack", [1, K], bf16, isOutput=False)
    v_d = nc.declare_dram_parameter("v_pack", [128, KT], bf16, isOutput=False)
    out_d = nc.declare_dram_parameter("out", [BL, S], f32, isOutput=True)

    with tile.TileContext(nc) as tc, ExitStack() as ctx:
        const = ctx.enter_context(tc.tile_pool(name="const", bufs=1))
        tp_pool = ctx.enter_context(tc.tile_pool(name="encT", bufs=6))
        tanh_pool = ctx.enter_context(tc.tile_pool(name="tanh", bufs=5))
        pre_ps = ctx.enter_context(tc.tile_pool(name="pre_ps", bufs=3, space="PSUM"))
        sc_ps = ctx.enter_context(tc.tile_pool(name="sc_ps", bufs=1, space="PSUM"))
        fin = ctx.enter_context(tc.tile_pool(name="fin", bufs=1))

        # ---- weights (host-packed) ----
        # w8 gates the first main matmul: first on the sync ring, in two
        # DMAs so they overlap (per-DMA rate is ~143 GB/s)
        w8 = const.tile([128, ET2, 2, K], f8)
        nc.sync.dma_start(w8[:, :2], w8_d[:, :2])
        nc.sync.dma_start(w8[:, 2:], w8_d[:, 2:])
        hT_bf = const.tile([128, KT * BL], bf16)
        nc.sync.dma_start(hT_bf[:], hT_d[:])
        b_attn_bf = const.tile([1, K], bf16)
        nc.sync.dma_start(b_attn_bf[:], b_d[:])
        v_bf = const.tile([128, KT], bf16)
        nc.sync.dma_start(v_bf[:], v_d[:])
        # wh per-kt chunks on the scalar ring (repacked kt-major on host)
        wh_bf = const.tile([128, KT, KT, 128], bf16)  # [p, kt, dt, c]

        ones_bf = const.tile([1, BL], bf16)
        nc.vector.memset(ones_bf[:], 1.0)

        def stage_cast(b, h):
            """f32 -> fp8 DRAM->DRAM SWDGE cast of one s-half, in 4 chunks
            round-robined over the SWDGE queues (the CCE conversion rate is
            the limiter, ~75 G elem/s aggregate; the 1MB fp8 write hides
            inside the 4MB read)."""
            for st in range(ST // SB):
                nc.gpsimd.dma_start(
                    enc8_d[b][h][st * 128:(st + 1) * 128, :],
                    enc_d[b, h * NB + st * 128:h * NB + (st + 1) * 128, :])

        def stage_tp(b, h):
            """DRAM->SBUF u16-pair xbar transpose of the fp8 bounce half:
            encT8h u16[p, et, s] = fp8 pair
            (enc[b, h*512 + s, et*256+2p], enc[b, h*512 + s, et*256+2p+1]).
            Every transpose is a full DMA-system barrier (Tile fences it
            against ALL in-flight DMAs, both directions), so the schedule
            strictly alternates cast(batch) / tp(batch)."""
            eh = tp_pool.tile([128, ET2, NB], u16, tag="encT",
                              name=f"encT_{b}_{h}")
            nc.sync.dma_start(eh[:], enc8_d[b][h][:].bitcast(u16),
                              transpose=True)
            return eh

        def rhs_view(eh, f8dt):
            return eh[:].bitcast(f8dt).rearrange("p et (s j) -> p et j s", j=2)

        hpb = const.tile([128, KT * BL], f32)  # col = kt*BL + b

        def emit_hp_kt(kt):
            # h_proj[k, b] = sum_d Wh[d, k]*hidden[b, d] + b_attn[k],
            # one k-tile at a time, interleaved into batch 0's MM stream
            hp_ps = pre_ps.tile([128, NB], f32, tag="pre", name=f"hp_ps{kt}",
                                bufs=7)
            for dt in range(KT):
                nc.tensor.matmul(
                    hp_ps[:, :BL],
                    wh_bf[:, kt, dt, :],
                    hT_bf[:, dt * BL:(dt + 1) * BL],
                    start=(dt == 0),
                    stop=False,
                )
            nc.tensor.matmul(
                hp_ps[:, :BL],
                b_attn_bf[:, kt * 128:(kt + 1) * 128],
                ones_bf[:],
                start=False,
                stop=True,
            )
            # ACT, not DVE: the DVE is busy with next-batch casts at b0
            # start and the first tanh would block behind them (gpsimd
            # cannot read PSUM)
            nc.scalar.copy(hpb[:, kt * BL:(kt + 1) * BL], hp_ps[:, :BL])

        # scores accumulate on PSUM rows 32q (col-group q = s-quarter q);
        # staged in flat4 rows 32q on SBUF, gathered to [b, S] chunks
        flat4 = fin.tile([97, BL * 256], f32)

        # softmax state
        mask_i = fin.tile([BL, S], i32)
        maskoff = fin.tile([BL, S], f32)
        scores = fin.tile([BL, S], f32)
        scm = fin.tile([BL, S], f32)
        negmax = fin.tile([BL, 1], f32)
        expv = fin.tile([BL, S], f32)
        rowsum = fin.tile([BL, 1], f32)
        recip = fin.tile([BL, 1], f32)
        outf = fin.tile([BL, S], f32)

        def emit_mask_prep():
            nc.scalar.dma_start(mask_i[:], mask_d[:])
            nc.vector.tensor_copy(maskoff[:], mask_i[:])
            nc.vector.tensor_scalar(
                maskoff[:], maskoff[:], -NEG, NEG, Alu.mult, Alu.add)

        def emit_softmax_chunk(b0, b1):
            # gather flat4 quarter-rows into [b, s] layout (HWDGE SBUF->SBUF)
            for q in range(4):
                nc.scalar.dma_start(
                    scores[b0:b1, q * 256:(q + 1) * 256],
                    flat4[32 * q:32 * q + 1, b0 * 256:b1 * 256])
            # compute ops always span [0, b1): DVE/ACT partition windows
            # must start at 0 (quadrant alignment); recomputing the already
            # finished low batches is idempotent and partition-parallel
            nc.vector.tensor_add(
                scm[:b1, :], scores[:b1, :], maskoff[:b1, :])
            nc.vector.tensor_reduce(
                negmax[:b1], scm[:b1, :], Ax.X, Alu.max, negate=True)
            nc.scalar.activation(
                expv[:b1, :], scm[:b1, :], Exp, bias=negmax[:b1],
                accum_out=rowsum[:b1])
            nc.vector.reciprocal(recip[:b1], rowsum[:b1])
            nc.vector.tensor_scalar_mul(
                outf[:b1, :], expv[:b1, :], recip[:b1])
            nc.scalar.dma_start(out_d[b0:b1, :], outf[b0:b1, :])

        # ---- prologue ----
        # wh chunks on the scalar ring (free until the gathers at the end)
        for ck in range(KT):
            # wh_d dims are [p, kt, (dt c)] after the host kt-major repack
            nc.scalar.dma_start(wh_bf[:, ck], wh_d[:, ck])
        encTs = {}

        def stage_tps(b):
            for h in range(SB):
                encTs[(b, h)] = stage_tp(b, h)

        # strict cast/tp alternation from the start: bulk-casting ahead
        # would only make the first transpose's fence wait for all of it
        stage_cast(0, 0)
        encTs[(0, 0)] = stage_tp(0, 0)
        stage_cast(0, 1)
        encTs[(0, 1)] = stage_tp(0, 1)
        stage_cast(1, 0)
        stage_cast(1, 1)

        # scores PSUM: ONE bank, halves alternated by batch parity
        scband = sc_ps.tile([128, 2, 256], f32, tag="sc", name="scband")

        def scq(b, q):
            return scband[32 * q:32 * q + 1, b % 2, :]

        carry = []  # [(b_prev, kt, th_tile, col_off, qs)] not yet emitted

        def emit_vdots(b, kt, th, col_off, qs, stop):
            for q in qs:
                nc.tensor.matmul(
                    scq(b, q),
                    v_bf[:, kt:kt + 1],
                    th[:, col_off + (q - qs[0]) * 256:
                       col_off + (q - qs[0] + 1) * 256],
                    start=(kt == 0), stop=stop,
                    tile_position=(0, 32 * q))
            if stop:
                # all MMs above before any copy: interleaving creates false
                # column-range WARs that bubble the PE ~850ns per quarter
                for q in qs:
                    nc.vector.tensor_copy(
                        flat4[32 * q:32 * q + 1, b * 256:(b + 1) * 256],
                        scq(b, q))

        def emit_carry(budget):
            while carry and budget > 0:
                b_p, kt, th, col_off, qs = carry.pop(0)
                emit_vdots(b_p, kt, th, col_off, qs, stop=(kt == KT - 1))
                budget -= 1

        # ---- main loop over local batches (software-pipelined) ----
        for b in range(BL):
            if b == 1:
                emit_mask_prep()

            if b == 0:
                # sb-major: start on the first transposed s-half immediately;
                # hp(kt) interleaved into the sb0 pass
                ths = {}
                for sb in range(SB):
                    rh = rhs_view(encTs.pop((0, sb)), f8)
                    for kt in range(KT):
                        pre = pre_ps.tile([128, NB], f32, tag="pre",
                                          name="preh", bufs=7)
                        for et in range(ET2):
                            nc.tensor.matmul(
                                pre[:],
                                w8[:, et, :, kt * 128:(kt + 1) * 128],
                                rh[:, et, :, :],
                                start=(et == 0),
                                stop=(et == ET2 - 1),
                                perf_mode=DR,
                            )
                        if sb == 0:
                            emit_hp_kt(kt)
                        th = tanh_pool.tile([128, NB], bf16, tag="thh",
                                            name="thh", bufs=5)
                        nc.scalar.activation(
                            th[:], pre[:], Tanh,
                            bias=hpb[:, kt * BL:kt * BL + 1],
                            scale=1.0 / WSCALE,
                        )
                        ths[(sb, kt)] = th
                        if kt >= 1:
                            emit_vdots(0, kt - 1, ths[(sb, kt - 1)], 0,
                                       (2 * sb, 2 * sb + 1), stop=False)
                    carry.append((0, KT - 1, ths[(sb, KT - 1)], 0,
                                  (2 * sb, 2 * sb + 1)))
            else:
                rhA = rhs_view(encTs.pop((b, 0)), f8)
                rhB = rhs_view(encTs.pop((b, 1)), f8)
                ths = {}
                for kt in range(KT):
                    pres = [pre_ps.tile([128, NB], f32, tag="pre",
                                        name=f"pre{sb}", bufs=7)
                            for sb in range(SB)]
                    for et in range(ET2):  # one LDWEIGHTS serves both sb
                        for sb, rh in ((0, rhA), (1, rhB)):
                            nc.tensor.matmul(
                                pres[sb][:],
                                w8[:, et, :, kt * 128:(kt + 1) * 128],
                                rh[:, et, :, :],
                                start=(et == 0),
                                stop=(et == ET2 - 1),
                                perf_mode=DR,
                            )
                    if kt < VLAG:
                        emit_carry(1)
                    th = tanh_pool.tile([128, SB * NB], bf16, tag="tanh",
                                        bufs=5)
                    for sb in range(SB):
                        nc.scalar.activation(
                            th[:, sb * NB:(sb + 1) * NB], pres[sb][:], Tanh,
                            bias=hpb[:, kt * BL + b:kt * BL + b + 1],
                            scale=1.0 / WSCALE,
                        )
                    ths[kt] = th
                    if kt >= VLAG:
                        emit_vdots(b, kt - VLAG, ths[kt - VLAG], 0,
                                   (0, 1, 2, 3), stop=False)
                for kt in range(KT - VLAG, KT):
                    carry.append((b, kt, ths[kt], 0, (0, 1, 2, 3)))
            # stage the next batches AFTER this batch's MMs: the HWDGE sem
            # pool recycles every ~9 DMAs, and an MM emitted after tp(b+1)
            # would have to wait CONSERVATIVELY on tp(b+1)'s count for its
            # tp(b) dependency (same pool semaphore) — a spurious stall
            if b + 1 < BL:
                stage_tps(b + 1)
            if b + 2 < BL:
                stage_cast(b + 2, 0)
                stage_cast(b + 2, 1)
            if b == 7:
                emit_softmax_chunk(0, 6)

        emit_carry(len(carry))
        emit_softmax_chunk(6, 8)

    if strip:
        _split_multi_waits(nc, mybir)
    return nc


def _split_multi_waits(nc, mybir):
    """Move extra semaphore waits onto standalone NoOps on the same engine.

    This walrus build encodes at most one sync-wait command per instruction,
    but Tile emits instructions with several (cross-engine RAW + WAR + DMA
    queue ordering). A NoOp carrying one wait, placed immediately before the
    instruction in the same engine's stream, is semantically identical: the
    engine's sequencer blocks on the NoOp's wait before dispatching the real
    instruction.
    """
    n = 0
    for fn in nc.m.functions:
        for blk in fn.blocks:
            insts = blk.instructions
            new = []
            changed = False
            for inst in insts:
                si = inst.sync_info
                if si is not None and si.on_wait and len(si.on_wait) > 1:
                    for w in list(si.on_wait)[:-1]:
                        n += 1
                        new.append(mybir.InstNoOp(
                            name=f"{inst.name}-sw{n}",
                            engine=inst.engine,
                            text_hint="split_wait",
                            bass_nofuse=True,
                            sync_info=mybir.SyncInfo(
                                on_wait=[w], on_update=[]),
                        ))
                    inst.sync_info = mybir.SyncInfo(
                        on_wait=[list(si.on_wait)[-1]],
                        on_update=list(si.on_update or []))
                    changed = True
                new.append(inst)
            if changed:
                blk.instructions = new


def get_nc(strip=True):
    key = ("nc", strip)
    if key not in _CACHE:
        _CACHE[key] = _build_bass(strip)
    return _CACHE[key]


def make_in_maps(hidden, encoder_outputs, mask, W_attn, b_attn, v):
    import ml_dtypes

    bf16 = ml_dtypes.bfloat16
    f8 = ml_dtypes.float8_e4m3

    W_attn = np.asarray(W_attn, dtype=np.float32)
    Wh, We = W_attn[:K], W_attn[K:]
    # wh_pack[p, kt, dt, c] = Wh[dt*128 + p, kt*128 + c]  (kt-major chunks)
    wh_pack = np.ascontiguousarray(
        Wh.reshape(KT, 128, KT, 128).transpose(1, 2, 0, 3).astype(bf16))
    # w8_pack[p, et, j, k] = 64 * We[et*256 + 2p + j, k]
    w8_pack = np.ascontiguousarray(
        (We * WSCALE).reshape(ET2, 128, 2, K).transpose(1, 0, 2, 3).astype(f8))
    b_pack = np.ascontiguousarray(
        np.asarray(b_attn, dtype=np.float32).reshape(1, K).astype(bf16))
    # v_pack[p, kt] = v[kt*128 + p]
    v_pack = np.ascontiguousarray(
        np.asarray(v, dtype=np.float32).reshape(KT, 128).T.astype(bf16))
    hidden = np.asarray(hidden, dtype=np.float32)

    in_maps = []
    for c in range(NCORES):
        sl = slice(c * BL, (c + 1) * BL)
        # hT_pack[p, dt*BL + b] = hidden[b, dt*128 + p]
        hT_pack = np.ascontiguousarray(
            hidden[sl].T.reshape(KT, 128, BL).transpose(1, 0, 2)
            .reshape(128, KT * BL).astype(bf16))
        in_maps.append({
            "encoder_outputs": np.ascontiguousarray(encoder_outputs[sl]),
            "mask": np.ascontiguousarray(np.asarray(mask[sl], dtype=np.int32)),
            "wh_pack": wh_pack,
            "w8_pack": w8_pack,
            "hT_pack": hT_pack,
            "b_pack": b_pack,
            "v_pack": v_pack,
        })
    return in_maps


def kernel(hidden, encoder_outputs, mask, W_attn, b_attn, v):
    from concourse.bass_utils import run_bass_kernel_spmd

    nc = get_nc()
    in_maps = make_in_maps(hidden, encoder_outputs, mask, W_attn, b_attn, v)
    res = run_bass_kernel_spmd(nc, in_maps, core_ids=list(range(NCORES)))
    return np.concatenate(
        [np.asarray(res.results[c]["out"], dtype=np.float32) for c in range(NCORES)],
        axis=0,
    )


# revision 26
# speedup vs baseline: 1.2063x; 1.0922x over previous
"""Bahdanau-attention scores kernel for Trainium2 (8 NeuronCores, SPMD).

Computation (per batch row b):
    pre[s, k] = hidden[b] @ Wh + enc[b, s] @ We + b_attn       (S=1024, E=K=1024)
    scores[s] = tanh(pre[s, :]) @ v
    out[b]    = softmax(where(mask[b]==0, -1e10, scores))      over s

Sharding: data-parallel over batch B=64 -> 8 batches per core; weights
replicated. No collectives.

Per-core structure (fp8 DoubleRow main matmul, bf16 elsewhere):
  - enc HBM->fp8-pair pipeline runs entirely on HWDGE rings + DVE, with NO
    SWDGE DMAs in flight: Tile fences xbar transposes against every
    in-flight SWDGE DMA (both directions, all 8 DMASW queues), which in the
    previous DRAM->DRAM SWDGE-cast design serialized cast(13.4us) ->
    tp-issue(4.2) -> tp(5.2) into the 23.9us/batch critical path.  Now:
      scalar ring: enc f32 DRAM->SBUF loads (natural [p=s%128, st, e])
      DVE:         tensor_copy f32 -> fp8e4 (adjacent-e pairs land in
                   adjacent bytes = the DoubleRow rhs pairing)
      sync ring:   fp8 SBUF->DRAM bounce store, then DRAM->SBUF xbar
                   transpose of the u16 pairs: encT8h[p, et, s]
    HWDGE traffic/batch = 4MB(load)+1MB(store)+1MB(tp) = 6MB at the
    ~358 GB/s per-core cap = 16.8us ~= the PE's ~17us/batch: true ridge.
  - one DRAM bounce tensor per (batch, half) so no coarse DRAM-range
    tracking can serialize different batches' stores/transposes.
  - main MM: pre[k, s] = sum_et lhsT(w8) @ rhs(encT8h), DoubleRow, one
    LDWEIGHTS per (et, kt) serving both s-halves.
  - ScalarE: tanh(psum/64 + (hidden@Wh + b_attn)[k]) -> SBUF bf16
  - hidden@Wh (hp) is interleaved per-kt into batch 0's loop (wh loaded as
    per-kt chunks) so it doesn't sit at the PE FIFO head blocking the
    first main MMs behind a 2MB weight load.
  - v-dot: 4 col-tiled PE matmuls (tile_position=(0,32q)), lag THREE
    k-tiles behind the main MMs (ACT falls ~1.5 groups behind the PE by
    batch end; lag 2 stalled the PE 0.5-1.5us per batch).  Last 3 k-tiles
    carried into the next batch, one k-tile per MM group, all 4 quarter
    MMs before the 4 flat4 copies (interleaving MMs and copies created
    false column-range WARs = 3x850ns PE bubbles).
  - softmax in chunks: batches 0-5 during iter 7, 6-7 at the end; gathers
    flat4->scores are HWDGE SBUF->SBUF (no accum; mask offset added on
    DVE), so they don't fence against transposes either.

Sync note: this walrus build encodes at most ONE semaphore wait per
instruction; _split_multi_waits() rewrites Tile's multi-wait instructions
into NoOp(wait) chains on the same engine.
"""

import sys

if "/opt/trn_rl_repo" not in sys.path:
    sys.path.insert(0, "/opt/trn_rl_repo")

from contextlib import ExitStack

import numpy as np

B, S, E, K = 64, 1024, 1024, 1024  # E = 2*ENC_HID, K = DEC_HID
NCORES = 8
BL = B // NCORES  # batches per core
NEG = -1e10
WSCALE = 64.0     # We quantization scale into E4M3 range

ET2 = E // 256  # 4 DoubleRow e-tiles (256-deep contraction each)
KT = K // 128   # 8 k-tiles
ST = S // 128   # 8 s-tiles
NB = 512        # matmul free-dim block (one s-half)
SB = S // NB    # 2 s-halves
VLAG = 3        # v-dot lag in k-tiles behind the main MMs

_CACHE = {}


def _build_bass(strip=True):
    from concourse import bass, mybir, tile

    f32 = mybir.dt.float32
    bf16 = mybir.dt.bfloat16
    f8 = mybir.dt.float8e4
    u16 = mybir.dt.uint16
    i32 = mybir.dt.int32
    Tanh = mybir.ActivationFunctionType.Tanh
    Exp = mybir.ActivationFunctionType.Exp
    Alu = mybir.AluOpType
    Ax = mybir.AxisListType
    DR = mybir.MatmulPerfMode.DoubleRow

    nc = bass.Bass()

    enc_d = nc.declare_dram_parameter("encoder_outputs", [BL, S, E], f32, isOutput=False)
    # one bounce tensor per (batch, half): no shared-tensor dep tracking
    enc8_d = [[nc.dram_tensor(f"enc8_{b}_{h}", [NB, E], f8) for h in range(SB)]
              for b in range(BL)]
    mask_d = nc.declare_dram_parameter("mask", [BL, S], i32, isOutput=False)
    wh_d = nc.declare_dram_parameter("wh_pack", [128, KT, K], bf16, isOutput=False)
    w8_d = nc.declare_dram_parameter("w8_pack", [128, ET2, 2, K], f8, isOutput=False)
    hT_d = nc.declare_dram_parameter("hT_pack", [128, KT * BL], bf16, isOutput=False)
    b_d = nc.declare_dram_parameter("b_pack", [1, K], bf16, isOutput=False)
    v_d = nc.declare_dram_parameter("v_pack", [128, KT], bf16, isOutput=False)
    out_d = nc.declare_dram_parameter("out", [BL, S], f32, isOutput=True)

    with tile.TileContext(nc) as tc, ExitStack() as ctx:
        const = ctx.enter_context(tc.tile_pool(name="const", bufs=1))
        tp_pool = ctx.enter_context(tc.tile_pool(name="encT", bufs=6))
        tanh_pool = ctx.enter_context(tc.tile_pool(name="tanh", bufs=5))
        pre_ps = ctx.enter_context(tc.tile_pool(name="pre_ps", bufs=3, space="PSUM"))
        sc_ps = ctx.enter_context(tc.tile_pool(name="sc_ps", bufs=1, space="PSUM"))
        fin = ctx.enter_context(tc.tile_pool(name="fin", bufs=1))

        # ---- weights (host-packed) ----
        # w8 gates the first main matmul: first on the sync ring, in two
        # DMAs so they overlap (per-DMA rate is ~143 GB/s)
        w8 = const.tile([128, ET2, 2, K], f8)
        nc.sync.dma_start(w8[:, :2], w8_d[:, :2])
        nc.sync.dma_start(w8[:, 2:], w8_d[:, 2:])
        hT_bf = const.tile([128, KT * BL], bf16)
        nc.sync.dma_start(hT_bf[:], hT_d[:])
        b_attn_bf = const.tile([1, K], bf16)
        nc.sync.dma_start(b_attn_bf[:], b_d[:])
        v_bf = const.tile([128, KT], bf16)
        nc.sync.dma_start(v_bf[:], v_d[:])
        # wh per-kt chunks on the scalar ring (repacked kt-major on host)
        wh_bf = const.tile([128, KT, KT, 128], bf16)  # [p, kt, dt, c]

        ones_bf = const.tile([1, BL], bf16)
        nc.vector.memset(ones_bf[:], 1.0)

        def stage_cast(b, h):
            """f32 -> fp8 DRAM->DRAM SWDGE cast of one s-half, in 4 chunks
            round-robined over the SWDGE queues (the CCE conversion rate is
            the limiter, ~75 G elem/s aggregate; the 1MB fp8 write hides
            inside the 4MB read)."""
            for st in range(ST // SB):
                nc.gpsimd.dma_start(
                    enc8_d[b][h][st * 128:(st + 1) * 128, :],
                    enc_d[b, h * NB + st * 128:h * NB + (st + 1) * 128, :])

        def stage_tp(b, h):
            """DRAM->SBUF u16-pair xbar transpose of the fp8 bounce half:
            encT8h u16[p, et, s] = fp8 pair
            (enc[b, h*512 + s, et*256+2p], enc[b, h*512 + s, et*256+2p+1]).
            Every transpose is a full DMA-system barrier (Tile fences it
            against ALL in-flight DMAs, both directions), so the schedule
            strictly alternates cast(batch) / tp(batch)."""
            eh = tp_pool.tile([128, ET2, NB], u16, tag="encT",
                              name=f"encT_{b}_{h}")
            nc.sync.dma_start(eh[:], enc8_d[b][h][:].bitcast(u16),
                              transpose=True)
            return eh

        def rhs_view(eh, f8dt):
            return eh[:].bitcast(f8dt).rearrange("p et (s j) -> p et j s", j=2)

        hpb = const.tile([128, KT * BL], f32)  # col = kt*BL + b

        def emit_hp_kt(kt):
            # h_proj[k, b] = sum_d Wh[d, k]*hidden[b, d] + b_attn[k],
            # one k-tile at a time, interleaved into batch 0's MM stream
            hp_ps = pre_ps.tile([128, NB], f32, tag="pre", name=f"hp_ps{kt}",
                                bufs=7)
            for dt in range(KT):
                nc.tensor.matmul(
                    hp_ps[:, :BL],
                    wh_bf[:, kt, dt, :],
                    hT_bf[:, dt * BL:(dt + 1) * BL],
                    start=(dt == 0),
                    stop=False,
                )
            nc.tensor.matmul(
                hp_ps[:, :BL],
                b_attn_bf[:, kt * 128:(kt + 1) * 128],
                ones_bf[:],
                start=False,
                stop=True,
            )
            # ACT, not DVE: the DVE is busy with next-batch casts at b0
            # start and the first tanh would block behind them (gpsimd
            # cannot read PSUM)
            nc.scalar.copy(hpb[:, kt * BL:(kt + 1) * BL], hp_ps[:, :BL])

        # scores accumulate on PSUM rows 32q (col-group q = s-quarter q);
        # staged in flat4 rows 32q on SBUF, gathered to [b, S] chunks
        flat4 = fin.tile([97, BL * 256], f32)

        # softmax state
        mask_i = fin.tile([BL, S], i32)
        maskoff = fin.tile([BL, S], f32)
        scores = fin.tile([BL, S], f32)
        scm = fin.tile([BL, S], f32)
        negmax = fin.tile([BL, 1], f32)
        expv = fin.tile([BL, S], f32)
        rowsum = fin.tile([BL, 1], f32)
        recip = fin.tile([BL, 1], f32)
        outf = fin.tile([BL, S], f32)

        def emit_mask_prep():
            nc.scalar.dma_start(mask_i[:], mask_d[:])
            nc.vector.tensor_copy(maskoff[:], mask_i[:])
            nc.vector.tensor_scalar(
                maskoff[:], maskoff[:], -NEG, NEG, Alu.mult, Alu.add)

        def emit_softmax_chunk(b0, b1):
            # gather flat4 quarter-rows into [b, s] layout (HWDGE SBUF->SBUF)
            for q in range(4):
                nc.scalar.dma_start(
                    scores[b0:b1, q * 256:(q + 1) * 256],
                    flat4[32 * q:32 * q + 1, b0 * 256:b1 * 256])
            # compute ops always span [0, b1): DVE/ACT partition windows
            # must start at 0 (quadrant alignment); recomputing the already
            # finished low batches is idempotent and partition-parallel
            nc.vector.tensor_add(
                scm[:b1, :], scores[:b1, :], maskoff[:b1, :])
            nc.vector.tensor_reduce(
                negmax[:b1], scm[:b1, :], Ax.X, Alu.max, negate=True)
            nc.scalar.activation(
                expv[:b1, :], scm[:b1, :], Exp, bias=negmax[:b1],
                accum_out=rowsum[:b1])
            nc.vector.reciprocal(recip[:b1], rowsum[:b1])
            nc.vector.tensor_scalar_mul(
                outf[:b1, :], expv[:b1, :], recip[:b1])
            nc.scalar.dma_start(out_d[b0:b1, :], outf[b0:b1, :])

        # ---- prologue ----
        # wh chunks on the scalar ring (free until the gathers at the end)
        for ck in range(KT):
            # wh_d dims are [p, kt, (dt c)] after the host kt-major repack
            nc.scalar.dma_start(wh_bf[:, ck], wh_d[:, ck])
        encTs = {}

        def stage_tps(b):
            for h in range(SB):
                encTs[(b, h)] = stage_tp(b, h)

        # strict cast/tp alternation from the start: bulk-casting ahead
        # would only make the first transpose's fence wait for all of it
        stage_cast(0, 0)
        encTs[(0, 0)] = stage_tp(0, 0)
        stage_cast(0, 1)
        encTs[(0, 1)] = stage_tp(0, 1)
        stage_cast(1, 0)
        stage_cast(1, 1)

        # scores PSUM: ONE bank, halves alternated by batch parity
        scband = sc_ps.tile([128, 2, 256], f32, tag="sc", name="scband")

        def scq(b, q):
            return scband[32 * q:32 * q + 1, b % 2, :]

        carry = []  # [(b_prev, kt, th_tile, col_off, qs)] not yet emitted

        def emit_vdots(b, kt, th, col_off, qs, stop):
            for q in qs:
                nc.tensor.matmul(
                    scq(b, q),
                    v_bf[:, kt:kt + 1],
                    th[:, col_off + (q - qs[0]) * 256:
                       col_off + (q - qs[0] + 1) * 256],
                    start=(kt == 0), stop=stop,
                    tile_position=(0, 32 * q))
            if stop:
                # all MMs above before any copy: interleaving creates false
                # column-range WARs that bubble the PE ~850ns per quarter
                for q in qs:
                    nc.vector.tensor_copy(
                        flat4[32 * q:32 * q + 1, b * 256:(b + 1) * 256],
                        scq(b, q))

        def emit_carry(budget):
            while carry and budget > 0:
                b_p, kt, th, col_off, qs = carry.pop(0)
                emit_vdots(b_p, kt, th, col_off, qs, stop=(kt == KT - 1))
                budget -= 1

        # ---- main loop over local batches (software-pipelined) ----
        for b in range(BL):
            if b == 1:
                emit_mask_prep()

            if b == 0:
                # sb-major: start on the first transposed s-half immediately;
                # hp(kt) interleaved into the sb0 pass
                ths = {}
                for sb in range(SB):
                    rh = rhs_view(encTs.pop((0, sb)), f8)
                    for kt in range(KT):
                        pre = pre_ps.tile([128, NB], f32, tag="pre",
                                          name="preh", bufs=7)
                        for et in range(ET2):
                            nc.tensor.matmul(
                                pre[:],
                                w8[:, et, :, kt * 128:(kt + 1) * 128],
                                rh[:, et, :, :],
                                start=(et == 0),
                                stop=(et == ET2 - 1),
                                perf_mode=DR,
                            )
                        if sb == 0:
                            emit_hp_kt(kt)
                        th = tanh_pool.tile([128, NB], bf16, tag="thh",
                                            name="thh", bufs=5)
                        nc.scalar.activation(
                            th[:], pre[:], Tanh,
                            bias=hpb[:, kt * BL:kt * BL + 1],
                            scale=1.0 / WSCALE,
                        )
                        ths[(sb, kt)] = th
                        if kt >= 1:
                            emit_vdots(0, kt - 1, ths[(sb, kt - 1)], 0,
                                       (2 * sb, 2 * sb + 1), stop=False)
                    carry.append((0, KT - 1, ths[(sb, KT - 1)], 0,
                                  (2 * sb, 2 * sb + 1)))
            else:
                rhA = rhs_view(encTs.pop((b, 0)), f8)
                rhB = rhs_view(encTs.pop((b, 1)), f8)
                ths = {}
                for kt in range(KT):
                    pres = [pre_ps.tile([128, NB], f32, tag="pre",
                                        name=f"pre{sb}", bufs=7)
                            for sb in range(SB)]
                    for et in range(ET2):  # one LDWEIGHTS serves both sb
                        for sb, rh in ((0, rhA), (1, rhB)):
                            nc.tensor.matmul(
                                pres[sb][:],
                                w8[:, et, :, kt * 128:(kt + 1) * 128],
                                rh[:, et, :, :],
                                start=(et == 0),
                                stop=(et == ET2 - 1),
                                perf_mode=DR,
                            )
                    if kt < VLAG:
                        emit_carry(1)
                    th = tanh_pool.tile([128, SB * NB], bf16, tag="tanh",
                                        bufs=5)
                    for sb in range(SB):
                        nc.scalar.activation(
                            th[:, sb * NB:(sb + 1) * NB], pres[sb][:], Tanh,
                            bias=hpb[:, kt * BL + b:kt * BL + b + 1],
                            scale=1.0 / WSCALE,
                        )
                    ths[kt] = th
                    if kt >= VLAG:
                        emit_vdots(b, kt - VLAG, ths[kt - VLAG], 0,
                                   (0, 1, 2, 3), stop=False)
                for kt in range(KT - VLAG, KT):
                    carry.append((b, kt, ths[kt], 0, (0, 1, 2, 3)))
            # stage the next batches AFTER this batch's MMs (readers of a
            # transpose-written tile conservatively wait on the last
            # transpose emitted before them), and casts BEFORE tps so the
            # fence alternation is ONE cast-block + ONE tp-block per batch:
            # casts(b+2) fence-wait tps(b) (done), run ~14us; tps(b+1)
            # fence-wait casts(b+2), run ~9us -> 23.4us steady period
            if b + 2 < BL:
                stage_cast(b + 2, 0)
                stage_cast(b + 2, 1)
            if b + 1 < BL:
                stage_tps(b + 1)
            if b == 7:
                emit_softmax_chunk(0, 6)

        emit_carry(len(carry))
        emit_softmax_chunk(6, 8)

    if strip:
        _split_multi_waits(nc, mybir)
    return nc


def _split_multi_waits(nc, mybir):
    """Move extra semaphore waits onto standalone NoOps on the same engine.

    This walrus build encodes at most one sync-wait command per instruction,
    but Tile emits instructions with several (cross-engine RAW + WAR + DMA
    queue ordering). A NoOp carrying one wait, placed immediately before the
    instruction in the same engine's stream, is semantically identical: the
    engine's sequencer blocks on the NoOp's wait before dispatching the real
    instruction.
    """
    n = 0
    for fn in nc.m.functions:
        for blk in fn.blocks:
            insts = blk.instructions
            new = []
            changed = False
            for inst in insts:
                si = inst.sync_info
                if si is not None and si.on_wait and len(si.on_wait) > 1:
                    for w in list(si.on_wait)[:-1]:
                        n += 1
                        new.append(mybir.InstNoOp(
                            name=f"{inst.name}-sw{n}",
                            engine=inst.engine,
                            text_hint="split_wait",
                            bass_nofuse=True,
                            sync_info=mybir.SyncInfo(
                                on_wait=[w], on_update=[]),
                        ))
                    inst.sync_info = mybir.SyncInfo(
                        on_wait=[list(si.on_wait)[-1]],
                        on_update=list(si.on_update or []))
                    changed = True
                new.append(inst)
            if changed:
                blk.instructions = new


def get_nc(strip=True):
    key = ("nc", strip)
    if key not in _CACHE:
        _CACHE[key] = _build_bass(strip)
    return _CACHE[key]


def make_in_maps(hidden, encoder_outputs, mask, W_attn, b_attn, v):
    import ml_dtypes

    bf16 = ml_dtypes.bfloat16
    f8 = ml_dtypes.float8_e4m3

    W_attn = np.asarray(W_attn, dtype=np.float32)
    Wh, We = W_attn[:K], W_attn[K:]
    # wh_pack[p, kt, dt, c] = Wh[dt*128 + p, kt*128 + c]  (kt-major chunks)
    wh_pack = np.ascontiguousarray(
        Wh.reshape(KT, 128, KT, 128).transpose(1, 2, 0, 3).astype(bf16))
    # w8_pack[p, et, j, k] = 64 * We[et*256 + 2p + j, k]
    w8_pack = np.ascontiguousarray(
        (We * WSCALE).reshape(ET2, 128, 2, K).transpose(1, 0, 2, 3).astype(f8))
    b_pack = np.ascontiguousarray(
        np.asarray(b_attn, dtype=np.float32).reshape(1, K).astype(bf16))
    # v_pack[p, kt] = v[kt*128 + p]
    v_pack = np.ascontiguousarray(
        np.asarray(v, dtype=np.float32).reshape(KT, 128).T.astype(bf16))
    hidden = np.asarray(hidden, dtype=np.float32)

    in_maps = []
    for c in range(NCORES):
        sl = slice(c * BL, (c + 1) * BL)
        # hT_pack[p, dt*BL + b] = hidden[b, dt*128 + p]
        hT_pack = np.ascontiguousarray(
            hidden[sl].T.reshape(KT, 128, BL).transpose(1, 0, 2)
            .reshape(128, KT * BL).astype(bf16))
        in_maps.append({
            "encoder_outputs": np.ascontiguousarray(encoder_outputs[sl]),
            "mask": np.ascontiguousarray(np.asarray(mask[sl], dtype=np.int32)),
            "wh_pack": wh_pack,
            "w8_pack": w8_pack,
            "hT_pack": hT_pack,
            "b_pack": b_pack,
            "v_pack": v_pack,
        })
    return in_maps


def kernel(hidden, encoder_outputs, mask, W_attn, b_attn, v):
    from concourse.bass_utils import run_bass_kernel_spmd

    nc = get_nc()
    in_maps = make_in_maps(hidden, encoder_outputs, mask, W_attn, b_attn, v)
    res = run_bass_kernel_spmd(nc, in_maps, core_ids=list(range(NCORES)))
    return np.concatenate(
        [np.asarray(res.results[c]["out"], dtype=np.float32) for c in range(NCORES)],
        axis=0,
    )


# revision 31
# speedup vs baseline: 1.3149x; 1.0900x over previous
"""Bahdanau-attention scores kernel for Trainium2 (8 NeuronCores, SPMD).

Computation (per batch row b):
    pre[s, k] = hidden[b] @ Wh + enc[b, s] @ We + b_attn       (S=1024, E=K=1024)
    scores[s] = tanh(pre[s, :]) @ v
    out[b]    = softmax(where(mask[b]==0, -1e10, scores))      over s

Sharding: data-parallel over batch B=64 -> 8 batches per core; weights
replicated. No collectives.

Per-core structure (fp8 DoubleRow main matmul, bf16 elsewhere):
  - enc HBM->fp8-pair pipeline runs entirely on HWDGE rings + DVE, with NO
    SWDGE DMAs in flight: Tile fences xbar transposes against every
    in-flight SWDGE DMA (both directions, all 8 DMASW queues), which in the
    previous DRAM->DRAM SWDGE-cast design serialized cast(13.4us) ->
    tp-issue(4.2) -> tp(5.2) into the 23.9us/batch critical path.  Now:
      scalar ring: enc f32 DRAM->SBUF loads (natural [p=s%128, st, e])
      DVE:         tensor_copy f32 -> fp8e4 (adjacent-e pairs land in
                   adjacent bytes = the DoubleRow rhs pairing)
      sync ring:   fp8 SBUF->DRAM bounce store, then DRAM->SBUF xbar
                   transpose of the u16 pairs: encT8h[p, et, s]
    HWDGE traffic/batch = 4MB(load)+1MB(store)+1MB(tp) = 6MB at the
    ~358 GB/s per-core cap = 16.8us ~= the PE's ~17us/batch: true ridge.
  - one DRAM bounce tensor per (batch, half) so no coarse DRAM-range
    tracking can serialize different batches' stores/transposes.
  - main MM: pre[k, s] = sum_et lhsT(w8) @ rhs(encT8h), DoubleRow, one
    LDWEIGHTS per (et, kt) serving both s-halves.
  - ScalarE: tanh(psum/64 + (hidden@Wh + b_attn)[k]) -> SBUF bf16
  - hidden@Wh (hp) is interleaved per-kt into batch 0's loop (wh loaded as
    per-kt chunks) so it doesn't sit at the PE FIFO head blocking the
    first main MMs behind a 2MB weight load.
  - v-dot: 4 col-tiled PE matmuls (tile_position=(0,32q)), lag THREE
    k-tiles behind the main MMs (ACT falls ~1.5 groups behind the PE by
    batch end; lag 2 stalled the PE 0.5-1.5us per batch).  Last 3 k-tiles
    carried into the next batch, one k-tile per MM group, all 4 quarter
    MMs before the 4 flat4 copies (interleaving MMs and copies created
    false column-range WARs = 3x850ns PE bubbles).
  - softmax in chunks: batches 0-5 during iter 7, 6-7 at the end; gathers
    flat4->scores are HWDGE SBUF->SBUF (no accum; mask offset added on
    DVE), so they don't fence against transposes either.

Sync note: this walrus build encodes at most ONE semaphore wait per
instruction; _split_multi_waits() rewrites Tile's multi-wait instructions
into NoOp(wait) chains on the same engine.
"""

import sys

if "/opt/trn_rl_repo" not in sys.path:
    sys.path.insert(0, "/opt/trn_rl_repo")

from contextlib import ExitStack

import numpy as np

B, S, E, K = 64, 1024, 1024, 1024  # E = 2*ENC_HID, K = DEC_HID
NCORES = 8
BL = B // NCORES  # batches per core
NEG = -1e10
WSCALE = 64.0     # We quantization scale into E4M3 range

ET2 = E // 256  # 4 DoubleRow e-tiles (256-deep contraction each)
KT = K // 128   # 8 k-tiles
ST = S // 128   # 8 s-tiles
NB = 512        # matmul free-dim block (one s-half)
SB = S // NB    # 2 s-halves
VLAG = 3        # v-dot lag in k-tiles behind the main MMs

_CACHE = {}


def _build_bass(strip=True):
    from concourse import bass, mybir, tile

    f32 = mybir.dt.float32
    bf16 = mybir.dt.bfloat16
    f8 = mybir.dt.float8e4
    u16 = mybir.dt.uint16
    i32 = mybir.dt.int32
    Tanh = mybir.ActivationFunctionType.Tanh
    Exp = mybir.ActivationFunctionType.Exp
    Alu = mybir.AluOpType
    Ax = mybir.AxisListType
    DR = mybir.MatmulPerfMode.DoubleRow

    nc = bass.Bass()

    enc_d = nc.declare_dram_parameter("encoder_outputs", [BL, S, E], f32, isOutput=False)
    # one bounce tensor per batch: no shared-tensor dep tracking
    enc8_d = [nc.dram_tensor(f"enc8_{b}", [S, E], f8) for b in range(BL)]
    mask_d = nc.declare_dram_parameter("mask", [BL, S], i32, isOutput=False)
    wh_d = nc.declare_dram_parameter("wh_pack", [128, KT, K], bf16, isOutput=False)
    w8_d = nc.declare_dram_parameter("w8_pack", [128, ET2, 2, K], f8, isOutput=False)
    hT_d = nc.declare_dram_parameter("hT_pack", [128, KT * BL], bf16, isOutput=False)
    b_d = nc.declare_dram_parameter("b_pack", [1, K], bf16, isOutput=False)
    v_d = nc.declare_dram_parameter("v_pack", [128, KT], bf16, isOutput=False)
    out_d = nc.declare_dram_parameter("out", [BL, S], f32, isOutput=True)

    with tile.TileContext(nc) as tc, ExitStack() as ctx:
        const = ctx.enter_context(tc.tile_pool(name="const", bufs=1))
        tp_pool = ctx.enter_context(tc.tile_pool(name="encT", bufs=3))
        tanh_pool = ctx.enter_context(tc.tile_pool(name="tanh", bufs=5))
        pre_ps = ctx.enter_context(tc.tile_pool(name="pre_ps", bufs=3, space="PSUM"))
        sc_ps = ctx.enter_context(tc.tile_pool(name="sc_ps", bufs=1, space="PSUM"))
        fin = ctx.enter_context(tc.tile_pool(name="fin", bufs=1))

        # ---- weights (host-packed) ----
        # w8 gates the first main matmul: first on the sync ring, in two
        # DMAs so they overlap (per-DMA rate is ~143 GB/s)
        w8 = const.tile([128, ET2, 2, K], f8)
        nc.sync.dma_start(w8[:, :2], w8_d[:, :2])
        nc.sync.dma_start(w8[:, 2:], w8_d[:, 2:])
        hT_bf = const.tile([128, KT * BL], bf16)
        nc.sync.dma_start(hT_bf[:], hT_d[:])
        b_attn_bf = const.tile([1, K], bf16)
        nc.sync.dma_start(b_attn_bf[:], b_d[:])
        v_bf = const.tile([128, KT], bf16)
        nc.sync.dma_start(v_bf[:], v_d[:])
        # wh per-kt chunks on the scalar ring (repacked kt-major on host)
        wh_bf = const.tile([128, KT, KT, 128], bf16)  # [p, kt, dt, c]

        ones_bf = const.tile([1, BL], bf16)
        nc.vector.memset(ones_bf[:], 1.0)

        def stage_cast(b):
            """f32 -> fp8 DRAM->DRAM SWDGE cast of one batch, in 8 chunks
            round-robined over the 8 SWDGE queues (the ~300 GB/s read side
            is the limiter; the 1MB fp8 write hides inside the 4MB read)."""
            for st in range(ST):
                nc.gpsimd.dma_start(
                    enc8_d[b][st * 128:(st + 1) * 128, :],
                    enc_d[b, st * 128:(st + 1) * 128, :])

        def stage_tp(b):
            """DRAM->SBUF u16-pair xbar transpose of the fp8 bounce:
            encT8 u16[p, et, s] = fp8 pair
            (enc[b, s, et*256+2p], enc[b, s, et*256+2p+1]).
            Every transpose is a full DMA-system barrier (Tile fences it
            against ALL in-flight DMAs, both directions), so the schedule
            strictly alternates cast(batch) / tp(batch) — ONE fence cycle
            per batch; splitting into halves costs a second ~5us hop."""
            eh = tp_pool.tile([128, ET2, S], u16, tag="encT",
                              name=f"encT_{b}")
            nc.sync.dma_start(eh[:], enc8_d[b][:].bitcast(u16),
                              transpose=True)
            return eh

        def rhs_view(eh, f8dt):
            return eh[:].bitcast(f8dt).rearrange("p et (s j) -> p et j s", j=2)

        hpb = const.tile([128, KT * BL], f32)  # col = kt*BL + b

        def emit_hp_kt(kt):
            # h_proj[k, b] = sum_d Wh[d, k]*hidden[b, d] + b_attn[k],
            # one k-tile at a time, interleaved into batch 0's MM stream
            hp_ps = pre_ps.tile([128, NB], f32, tag="pre", name=f"hp_ps{kt}",
                                bufs=7)
            for dt in range(KT):
                nc.tensor.matmul(
                    hp_ps[:, :BL],
                    wh_bf[:, kt, dt, :],
                    hT_bf[:, dt * BL:(dt + 1) * BL],
                    start=(dt == 0),
                    stop=False,
                )
            nc.tensor.matmul(
                hp_ps[:, :BL],
                b_attn_bf[:, kt * 128:(kt + 1) * 128],
                ones_bf[:],
                start=False,
                stop=True,
            )
            # ACT, not DVE: the DVE is busy with next-batch casts at b0
            # start and the first tanh would block behind them (gpsimd
            # cannot read PSUM)
            nc.scalar.copy(hpb[:, kt * BL:(kt + 1) * BL], hp_ps[:, :BL])

        # scores accumulate on PSUM rows 32q (col-group q = s-quarter q);
        # staged in flat4 rows 32q on SBUF, gathered to [b, S] chunks
        flat4 = fin.tile([97, BL * 256], f32)

        # softmax state
        mask_i = fin.tile([BL, S], i32)
        maskoff = fin.tile([BL, S], f32)
        scores = fin.tile([BL, S], f32)
        scm = fin.tile([BL, S], f32)
        negmax = fin.tile([BL, 1], f32)
        expv = fin.tile([BL, S], f32)
        rowsum = fin.tile([BL, 1], f32)
        recip = fin.tile([BL, 1], f32)
        outf = fin.tile([BL, S], f32)

        def emit_mask_prep():
            nc.scalar.dma_start(mask_i[:], mask_d[:])
            nc.vector.tensor_copy(maskoff[:], mask_i[:])
            nc.vector.tensor_scalar(
                maskoff[:], maskoff[:], -NEG, NEG, Alu.mult, Alu.add)

        def emit_softmax_chunk(b0, b1):
            # gather flat4 quarter-rows into [b, s] layout (HWDGE SBUF->SBUF)
            for q in range(4):
                nc.scalar.dma_start(
                    scores[b0:b1, q * 256:(q + 1) * 256],
                    flat4[32 * q:32 * q + 1, b0 * 256:b1 * 256])
            # compute ops always span [0, b1): DVE/ACT partition windows
            # must start at 0 (quadrant alignment); recomputing the already
            # finished low batches is idempotent and partition-parallel
            nc.vector.tensor_add(
                scm[:b1, :], scores[:b1, :], maskoff[:b1, :])
            nc.vector.tensor_reduce(
                negmax[:b1], scm[:b1, :], Ax.X, Alu.max, negate=True)
            nc.scalar.activation(
                expv[:b1, :], scm[:b1, :], Exp, bias=negmax[:b1],
                accum_out=rowsum[:b1])
            nc.vector.reciprocal(recip[:b1], rowsum[:b1])
            nc.vector.tensor_scalar_mul(
                outf[:b1, :], expv[:b1, :], recip[:b1])
            nc.scalar.dma_start(out_d[b0:b1, :], outf[b0:b1, :])

        # ---- prologue ----
        # wh chunks on the scalar ring (free until the gathers at the end)
        for ck in range(KT):
            # wh_d dims are [p, kt, (dt c)] after the host kt-major repack
            nc.scalar.dma_start(wh_bf[:, ck], wh_d[:, ck])
        encTs = {}

        # strict cast/tp alternation from the start: bulk-casting ahead
        # would only make the first transpose's fence wait for all of it
        stage_cast(0)
        encTs[0] = stage_tp(0)
        stage_cast(1)

        # scores PSUM: ONE bank, halves alternated by batch parity
        scband = sc_ps.tile([128, 2, 256], f32, tag="sc", name="scband")

        def scq(b, q):
            return scband[32 * q:32 * q + 1, b % 2, :]

        carry = []  # [(b_prev, kt, th_tile, col_off, qs)] not yet emitted

        def emit_vdots(b, kt, th, col_off, qs, stop):
            for q in qs:
                nc.tensor.matmul(
                    scq(b, q),
                    v_bf[:, kt:kt + 1],
                    th[:, col_off + (q - qs[0]) * 256:
                       col_off + (q - qs[0] + 1) * 256],
                    start=(kt == 0), stop=stop,
                    tile_position=(0, 32 * q))
            if stop:
                # all MMs above before any copy: interleaving creates false
                # column-range WARs that bubble the PE ~850ns per quarter
                for q in qs:
                    nc.vector.tensor_copy(
                        flat4[32 * q:32 * q + 1, b * 256:(b + 1) * 256],
                        scq(b, q))

        def emit_carry(budget):
            while carry and budget > 0:
                b_p, kt, th, col_off, qs = carry.pop(0)
                emit_vdots(b_p, kt, th, col_off, qs, stop=(kt == KT - 1))
                budget -= 1

        # ---- main loop over local batches (software-pipelined) ----
        for b in range(BL):
            if b == 1:
                emit_mask_prep()

            rhv = rhs_view(encTs.pop(b), f8)
            ths = {}
            for kt in range(KT):
                pres = [pre_ps.tile([128, NB], f32, tag="pre",
                                    name=f"pre{sb}", bufs=7)
                        for sb in range(SB)]
                for et in range(ET2):  # one LDWEIGHTS serves both sb
                    for sb in range(SB):
                        nc.tensor.matmul(
                            pres[sb][:],
                            w8[:, et, :, kt * 128:(kt + 1) * 128],
                            rhv[:, et, :, sb * NB:(sb + 1) * NB],
                            start=(et == 0),
                            stop=(et == ET2 - 1),
                            perf_mode=DR,
                        )
                if b == 0:
                    # hidden@Wh one k-tile at a time, off the startup
                    # critical path (wh streams in per-kt chunks)
                    emit_hp_kt(kt)
                elif kt < VLAG:
                    emit_carry(1)
                th = tanh_pool.tile([128, SB * NB], bf16, tag="tanh",
                                    bufs=5)
                for sb in range(SB):
                    nc.scalar.activation(
                        th[:, sb * NB:(sb + 1) * NB], pres[sb][:], Tanh,
                        bias=hpb[:, kt * BL + b:kt * BL + b + 1],
                        scale=1.0 / WSCALE,
                    )
                ths[kt] = th
                if kt >= VLAG:
                    emit_vdots(b, kt - VLAG, ths[kt - VLAG], 0,
                               (0, 1, 2, 3), stop=False)
            for kt in range(KT - VLAG, KT):
                carry.append((b, kt, ths[kt], 0, (0, 1, 2, 3)))
            # stage the next batches AFTER this batch's MMs (readers of a
            # transpose-written tile conservatively wait on the last
            # transpose emitted before them), and casts BEFORE tps so the
            # fence alternation is ONE cast-block + ONE tp-block per batch:
            # casts(b+2) fence-wait tps(b) (done), run ~14us; tps(b+1)
            # fence-wait casts(b+2), run ~9us -> ~24us steady period
            if b + 2 < BL:
                stage_cast(b + 2)
            if b + 1 < BL:
                encTs[b + 1] = stage_tp(b + 1)
            if b == 7:
                emit_softmax_chunk(0, 6)

        emit_carry(len(carry))
        emit_softmax_chunk(6, 8)

    if strip:
        _split_multi_waits(nc, mybir)
    return nc


def _split_multi_waits(nc, mybir):
    """Move extra semaphore waits onto standalone NoOps on the same engine.

    This walrus build encodes at most one sync-wait command per instruction,
    but Tile emits instructions with several (cross-engine RAW + WAR + DMA
    queue ordering). A NoOp carrying one wait, placed immediately before the
    instruction in the same engine's stream, is semantically identical: the
    engine's sequencer blocks on the NoOp's wait before dispatching the real
    instruction.
    """
    n = 0
    for fn in nc.m.functions:
        for blk in fn.blocks:
            insts = blk.instructions
            new = []
            changed = False
            for inst in insts:
                si = inst.sync_info
                if si is not None and si.on_wait and len(si.on_wait) > 1:
                    for w in list(si.on_wait)[:-1]:
                        n += 1
                        new.append(mybir.InstNoOp(
                            name=f"{inst.name}-sw{n}",
                            engine=inst.engine,
                            text_hint="split_wait",
                            bass_nofuse=True,
                            sync_info=mybir.SyncInfo(
                                on_wait=[w], on_update=[]),
                        ))
                    inst.sync_info = mybir.SyncInfo(
                        on_wait=[list(si.on_wait)[-1]],
                        on_update=list(si.on_update or []))
                    changed = True
                new.append(inst)
            if changed:
                blk.instructions = new


def get_nc(strip=True):
    key = ("nc", strip)
    if key not in _CACHE:
        _CACHE[key] = _build_bass(strip)
    return _CACHE[key]


def make_in_maps(hidden, encoder_outputs, mask, W_attn, b_attn, v):
    import ml_dtypes

    bf16 = ml_dtypes.bfloat16
    f8 = ml_dtypes.float8_e4m3

    W_attn = np.asarray(W_attn, dtype=np.float32)
    Wh, We = W_attn[:K], W_attn[K:]
    # wh_pack[p, kt, dt, c] = Wh[dt*128 + p, kt*128 + c]  (kt-major chunks)
    wh_pack = np.ascontiguousarray(
        Wh.reshape(KT, 128, KT, 128).transpose(1, 2, 0, 3).astype(bf16))
    # w8_pack[p, et, j, k] = 64 * We[et*256 + 2p + j, k]
    w8_pack = np.ascontiguousarray(
        (We * WSCALE).reshape(ET2, 128, 2, K).transpose(1, 0, 2, 3).astype(f8))
    b_pack = np.ascontiguousarray(
        np.asarray(b_attn, dtype=np.float32).reshape(1, K).astype(bf16))
    # v_pack[p, kt] = v[kt*128 + p]
    v_pack = np.ascontiguousarray(
        np.asarray(v, dtype=np.float32).reshape(KT, 128).T.astype(bf16))
    hidden = np.asarray(hidden, dtype=np.float32)

    in_maps = []
    for c in range(NCORES):
        sl = slice(c * BL, (c + 1) * BL)
        # hT_pack[p, dt*BL + b] = hidden[b, dt*128 + p]
        hT_pack = np.ascontiguousarray(
            hidden[sl].T.reshape(KT, 128, BL).transpose(1, 0, 2)
            .reshape(128, KT * BL).astype(bf16))
        in_maps.append({
            "encoder_outputs": np.ascontiguousarray(encoder_outputs[sl]),
            "mask": np.ascontiguousarray(np.asarray(mask[sl], dtype=np.int32)),
            "wh_pack": wh_pack,
            "w8_pack": w8_pack,
            "hT_pack": hT_pack,
            "b_pack": b_pack,
            "v_pack": v_pack,
        })
    return in_maps


def kernel(hidden, encoder_outputs, mask, W_attn, b_attn, v):
    from concourse.bass_utils import run_bass_kernel_spmd

    nc = get_nc()
    in_maps = make_in_maps(hidden, encoder_outputs, mask, W_attn, b_attn, v)
    res = run_bass_kernel_spmd(nc, in_maps, core_ids=list(range(NCORES)))
    return np.concatenate(
        [np.asarray(res.results[c]["out"], dtype=np.float32) for c in range(NCORES)],
        axis=0,
    )
